# revision 13
# baseline (speedup 1.0000x reference)
"""DeepKoopman Trainium2 kernel (8-core data-parallel).

Strategy (per core, 128-batch shard, all f32):
- Everything on-chip runs "T-form": features on SBUF partitions, tokens on the
  free dim. Each 512-token chunk = one batch row (S=512). Two half-chains
  share the PE array via tile_position inference: batch A uses partitions
  0:64, batch B partitions 64:128, so paired matmuls run on disjoint
  row/col strips concurrently.
- x arrives host-pre-transposed ([pair, 64, 512]: rows 0:32 even batch x^T,
  rows 32:64 odd) so input loads are single contiguous full-bandwidth DMAs;
  enc1 for batch B runs at tile position (32, 64).
- encoder: h1T = relu(enc_w1 @ xT + b1) (ACT per-partition bias),
  zT = enc_w2 @ h1T + b2 (DVE tensor_scalar add).
- The two decoder hidden layers (on z and on z_dyn) are ONE matmul per
  batch: lhsT = [dec_w1^T | (dec_w1 @ K_w)^T] (M=128) so z_dynT is never
  materialized.
- Token-major outputs come from "flip" matmuls (lhsT = activationT slice,
  stationary; rhs = small weights). z and z_dyn share one flip via
  rhs = [enc_w2^T | (enc_w2^T K_w^T)], bias [enc_b2 | enc_b2 @ K_w^T].
  CRITICAL HW RULE: matmuls that can run concurrently (disjoint PE row
  strips) must write DIFFERENT PSUM BANKS - same-bank concurrent drains
  lock up the device. Tiles are therefore bank-padded and row-strip
  streams get separate psum tiles.
- Bias for token-major outputs is added by DVE tensor_tensor with
  broadcast tiles built once via K=1 ones-matmuls.
- The P-step Koopman rollout is de-serialized with K-power matrices
  KPOW[:, 64(t-1):64t] = (K^T)^t built by doubling, then decoded like the
  main path; x_pred staging is chunk-major (one batch per partition,
  2KB-contiguous DRAM runs).
"""

import numpy as np

B, S, D, L, H, NCORES = 1024, 512, 32, 64, 64, 8
BS = B // NCORES          # 128 batches per core
NPAIR = BS // 2           # 64 pairs

_CACHE = {}


def _build(P, npair=NPAIR, do_main=True, do_pred=True):
    import concourse.mybir as mybir
    import concourse.tile as tile
    from concourse import bacc

    F32 = mybir.dt.float32
    AF = mybir.ActivationFunctionType
    ALU = mybir.AluOpType

    assert P % 16 == 0
    NTB = P // 16  # pred time blocks

    nc = bacc.Bacc("TRN2", target_bir_lowering=False, debug=False,
                   enable_asserts=True, num_devices=NCORES)

    xt_d = nc.dram_tensor("xt", [npair, 64, S], F32, kind="ExternalInput").ap()
    ew1_d = nc.dram_tensor("enc_w1", [H, D], F32, kind="ExternalInput").ap()
    eb1_d = nc.dram_tensor("enc_b1", [H], F32, kind="ExternalInput").ap()
    ew2_d = nc.dram_tensor("enc_w2", [L, H], F32, kind="ExternalInput").ap()
    eb2_d = nc.dram_tensor("enc_b2", [L], F32, kind="ExternalInput").ap()
    dw1_d = nc.dram_tensor("dec_w1", [H, L], F32, kind="ExternalInput").ap()
    db1_d = nc.dram_tensor("dec_b1", [H], F32, kind="ExternalInput").ap()
    dw2_d = nc.dram_tensor("dec_w2", [D, H], F32, kind="ExternalInput").ap()
    db2_d = nc.dram_tensor("dec_b2", [D], F32, kind="ExternalInput").ap()
    kw_d = nc.dram_tensor("K_w", [L, L], F32, kind="ExternalInput").ap()

    xrec_d = nc.dram_tensor("x_rec", [2 * npair, S, D], F32, kind="ExternalOutput").ap()
    xdyn_d = nc.dram_tensor("x_dyn", [2 * npair, S, D], F32, kind="ExternalOutput").ap()
    xprd_d = nc.dram_tensor("x_pred", [2 * npair, P, D], F32, kind="ExternalOutput").ap()
    z_d = nc.dram_tensor("z", [2 * npair, S, L], F32, kind="ExternalOutput").ap()
    zdyn_d = nc.dram_tensor("z_dyn", [2 * npair, S, L], F32, kind="ExternalOutput").ap()

    LO, HI = slice(0, 64), slice(64, 128)

    with tile.TileContext(nc) as tc:
        with tc.tile_pool(name="const", bufs=1) as cp:
            # ---------------- weights / biases / constants ----------------
            ew1t = cp.tile([128, H], F32)    # enc_w1^T at rows 0:32 and 32:64
            ew2s = cp.tile([128, H], F32)    # enc_w2 straight (lo only)
            dw1t = cp.tile([128, H], F32)    # dec_w1^T lo/hi
            dw2t = cp.tile([128, D], F32)    # dec_w2^T lo/hi
            kwt = cp.tile([128, L], F32)     # K_w^T lo/hi
            kst = cp.tile([128, L], F32)     # K_w straight lo/hi
            nc.sync.dma_start(ew1t[0:32, :], ew1_d.rearrange("a b -> b a"))
            nc.sync.dma_start(ew1t[32:64, :], ew1_d.rearrange("a b -> b a"))
            nc.sync.dma_start(ew2s[0:64, :], ew2_d[:, :])
            nc.sync.dma_start(dw1t[LO, :], dw1_d.rearrange("a b -> b a"))
            nc.sync.dma_start(dw1t[HI, :], dw1_d.rearrange("a b -> b a"))
            nc.sync.dma_start(dw2t[LO, :], dw2_d.rearrange("a b -> b a"))
            nc.sync.dma_start(dw2t[HI, :], dw2_d.rearrange("a b -> b a"))
            nc.sync.dma_start(kwt[LO, :], kw_d.rearrange("a b -> b a"))
            nc.sync.dma_start(kwt[HI, :], kw_d.rearrange("a b -> b a"))
            nc.sync.dma_start(kst[LO, :], kw_d[:, :])
            nc.sync.dma_start(kst[HI, :], kw_d[:, :])

            b1e = cp.tile([128, 1], F32)     # enc_b1 lo/hi (per-partition)
            b2e = cp.tile([128, 1], F32)     # enc_b2 lo/hi
            b1d = cp.tile([128, 1], F32)     # dec_b1 lo/hi
            for t_, src in ((b1e, eb1_d), (b2e, eb2_d), (b1d, db1_d)):
                nc.sync.dma_start(t_[LO, 0:1], src.rearrange("(a b) -> a b", b=1))
                nc.sync.dma_start(t_[HI, 0:1], src.rearrange("(a b) -> a b", b=1))

            # merged z/z_dyn flip weights [enc_w2^T | EK], EK[h,l'] = sum_l enc_w2[l,h] K^T[l,l']
            zzdw = cp.tile([128, 128], F32)
            # merged decoder-hidden weights [dec_w1^T | (dec_w1 @ K_w)^T]
            dwm = cp.tile([128, 128], F32)
            nc.sync.dma_start(zzdw[LO, 0:64], ew2_d.rearrange("a b -> b a"))
            nc.sync.dma_start(zzdw[HI, 0:64], ew2_d.rearrange("a b -> b a"))

            ones = cp.tile([128, 128], F32)
            nc.gpsimd.memset(ones[0:1, :], 1.0)
            zzd_row = cp.tile([128, 512], F32)  # 4x [enc_b2 | enc_b2 @ K_w^T]
            b2d_row = cp.tile([128, 512], F32)  # 16x dec_b2
            for r in range(4):
                nc.sync.dma_start(zzd_row[0:1, 128 * r:128 * r + 64],
                                  eb2_d.rearrange("(a b) -> a b", a=1))
            for r in range(16):
                nc.sync.dma_start(b2d_row[0:1, 32 * r:32 * r + 32],
                                  db2_d.rearrange("(a b) -> a b", a=1))
            zzd_rep = cp.tile([128, 512], F32)
            b2d_rep = cp.tile([128, 512], F32)
            bzrow = cp.tile([128, 64], F32)

            kpow = cp.tile([128, 64 * P], F32)   # (K^T)^t at [:, 64(t-1):64t]
            zlast = cp.tile([128, L], F32)       # col j = z[:,-1,:] of pair j
            qcur = cp.tile([128, L], F32)        # K^m (doubling helper)
            qnxt = cp.tile([128, L], F32)

            with tc.tile_pool(name="sps", bufs=2, space="PSUM") as sps:
                for base in (0, 64):
                    s_ = slice(base, base + 64)
                    ekp = sps.tile([128, 512], F32, tag="s")
                    nc.tensor.matmul(ekp[s_, 0:64], ew2s[0:64, :], kwt[0:64, :],
                                     start=True, stop=True, tile_position=(0, base))
                    nc.scalar.copy(zzdw[s_, 64:128], ekp[s_, 0:64])
                    wfp = sps.tile([128, 512], F32, tag="s")
                    nc.tensor.matmul(wfp[s_, 0:64], kst[s_, :], dw1t[s_, :],
                                     start=True, stop=True)
                    nc.scalar.copy(dwm[s_, 64:128], wfp[s_, 0:64])
                nc.vector.tensor_copy(dwm[LO, 0:64], dw1t[LO, :])
                nc.vector.tensor_copy(dwm[HI, 0:64], dw1t[HI, :])

                # bzd row = enc_b2 @ K_w^T
                bzp = sps.tile([128, 512], F32, tag="s")
                nc.tensor.matmul(bzp[0:1, 0:64], b2e[LO, 0:1], kwt[LO, :],
                                 start=True, stop=True)
                nc.vector.tensor_copy(bzrow[0:1, :], bzp[0:1, 0:64])
                for r in range(4):
                    nc.vector.tensor_copy(zzd_row[0:1, 128 * r + 64:128 * r + 128],
                                          bzrow[0:1, :])

                # broadcast bias rows to all partitions via K=1 ones matmuls
                bp = sps.tile([128, 512], F32, tag="s")
                nc.tensor.matmul(bp[:, :], ones[0:1, 0:128], zzd_row[0:1, :],
                                 start=True, stop=True)
                nc.vector.tensor_copy(zzd_rep[:, :], bp[:, :])
                bp2 = sps.tile([128, 512], F32, tag="s")
                nc.tensor.matmul(bp2[:, :], ones[0:1, 0:128], b2d_row[0:1, :],
                                 start=True, stop=True)
                nc.vector.tensor_copy(b2d_rep[:, :], bp2[:, :])

                # ---------------- K powers ----------------
                nc.vector.tensor_copy(kpow[:, 0:64], kwt[:, :])  # P_1
                for t in range(1, 8):  # P_2..P_8 (serial)
                    pp = sps.tile([128, 512], F32, tag="s")
                    for base in (0, 64):
                        s_ = slice(base, base + 64)
                        nc.tensor.matmul(pp[s_, 0:64], kst[s_, :],
                                         kpow[s_, 64 * (t - 1):64 * t],
                                         start=True, stop=True)
                    nc.vector.tensor_copy(kpow[:, 64 * t:64 * t + 64], pp[:, 0:64])
                nc.vector.tensor_copy(qcur[:, :], kst[:, :])  # Q_1
                for m in (1, 2, 4):   # Q_2, Q_4, Q_8
                    qp = sps.tile([128, 512], F32, tag="s")
                    for base in (0, 64):
                        s_ = slice(base, base + 64)
                        nc.tensor.matmul(qp[s_, 0:64],
                                         kpow[s_, 64 * (m - 1):64 * m],
                                         qcur[s_, :], start=True, stop=True)
                    nc.vector.tensor_copy(qnxt[:, :], qp[:, 0:64])
                    qcur, qnxt = qnxt, qcur
                m = 8
                while m < P:  # S_{m+1..2m} = mm(lhsT=Q_m, rhs=S_{1..m})
                    for ch in range(max(1, (64 * m) // 512)):
                        n0 = 512 * ch if 64 * m > 512 else 0
                        nn = min(512, 64 * m)
                        dp = sps.tile([128, 512], F32, tag="s")
                        for base in (0, 64):
                            s_ = slice(base, base + 64)
                            nc.tensor.matmul(dp[s_, 0:nn], qcur[s_, :],
                                             kpow[s_, n0:n0 + nn],
                                             start=True, stop=True)
                        nc.vector.tensor_copy(kpow[:, 64 * m + n0:64 * m + n0 + nn],
                                              dp[:, 0:nn])
                    if 2 * m < P:
                        qp = sps.tile([128, 512], F32, tag="s")
                        for base in (0, 64):
                            s_ = slice(base, base + 64)
                            nc.tensor.matmul(qp[s_, 0:64],
                                             kpow[s_, 64 * (m - 1):64 * m],
                                             qcur[s_, :], start=True, stop=True)
                        nc.vector.tensor_copy(qnxt[:, :], qp[:, 0:64])
                        qcur, qnxt = qnxt, qcur
                    m *= 2

            # ---------------- main loop over batch pairs ----------------
            # PSUM budget (8 banks): mm x2, hA, hB, ztmA, ztmB, xr, xd
            with tc.tile_pool(name="work", bufs=2) as wp_pool, \
                 tc.tile_pool(name="xin", bufs=3) as xin_pool, \
                 tc.tile_pool(name="stage", bufs=3) as st_pool, \
                 tc.tile_pool(name="mmp", bufs=2, space="PSUM") as mm_pool, \
                 tc.tile_pool(name="hp", bufs=1, space="PSUM") as h_pool, \
                 tc.tile_pool(name="tmp", bufs=1, space="PSUM") as tm_pool:

                for j in range(npair if do_main else 0):
                    bA = 2 * j
                    xT = xin_pool.tile([128, S], F32, tag="xT")
                    nc.sync.dma_start(xT[0:64, :], xt_d[j])

                    pre1 = mm_pool.tile([128, S], F32, tag="mm")
                    nc.tensor.matmul(pre1[LO, :], ew1t[0:32, :], xT[0:32, :],
                                     start=True, stop=True)
                    nc.tensor.matmul(pre1[HI, :], ew1t[32:64, :], xT[32:64, :],
                                     start=True, stop=True)
                    h1T = wp_pool.tile([128, S], F32, tag="h1T")
                    nc.scalar.activation(h1T[:, :], pre1[:, :], AF.Relu, bias=b1e[:, 0:1])

                    zp = mm_pool.tile([128, S], F32, tag="mm")
                    nc.tensor.matmul(zp[LO, :], zzdw[LO, 0:64], h1T[LO, :], start=True, stop=True)
                    nc.tensor.matmul(zp[HI, :], zzdw[HI, 0:64], h1T[HI, :], start=True, stop=True)
                    zT = wp_pool.tile([128, S], F32, tag="zT")
                    nc.vector.tensor_scalar(zT[:, :], zp[:, :], b2e[:, 0:1], None, op0=ALU.add)

                    # merged decoder hidden: rows 0:64 = h1(z), 64:128 = h1(z_dyn)
                    hA = h_pool.tile([128, S], F32, tag="hA")
                    hB = h_pool.tile([128, S], F32, tag="hB")
                    nc.tensor.matmul(hA[:, :], dwm[LO, :], zT[LO, :], start=True, stop=True)
                    nc.tensor.matmul(hB[:, :], dwm[HI, :], zT[HI, :], start=True, stop=True)
                    h1A = wp_pool.tile([128, S], F32, tag="h1A")
                    h1B = wp_pool.tile([128, S], F32, tag="h1B")
                    nc.scalar.activation(h1A[:, :], hA[:, :], AF.Relu, bias=b1d[:, 0:1])
                    nc.scalar.activation(h1B[:, :], hB[:, :], AF.Relu, bias=b1d[:, 0:1])

                    # ---- flip-mms (token-major) ----
                    ztmA = tm_pool.tile([128, 512], F32, tag="ztmA")
                    ztmB = tm_pool.tile([128, 512], F32, tag="ztmB")
                    for c in range(4):
                        cs = slice(128 * c, 128 * c + 128)
                        nc.tensor.matmul(ztmA[:, 128 * c:128 * c + 128],
                                         h1T[LO, cs], zzdw[LO, :], start=True, stop=True)
                        nc.tensor.matmul(ztmB[:, 128 * c:128 * c + 128],
                                         h1T[HI, cs], zzdw[HI, :], start=True, stop=True)
                    # xr: batches A+B from rows 0:64 of h1A/h1B (same strip -> same bank ok)
                    # xd: from rows 64:128 (different strip -> own bank)
                    xr = tm_pool.tile([128, 512], F32, tag="xr")   # bank-padded, use 0:256
                    xd = tm_pool.tile([128, 512], F32, tag="xd")
                    for c in range(4):
                        cs = slice(128 * c, 128 * c + 128)
                        nc.tensor.matmul(xr[:, 32 * c:32 * c + 32],
                                         h1A[LO, cs], dw2t[LO, :], start=True, stop=True)
                        nc.tensor.matmul(xr[:, 128 + 32 * c:128 + 32 * c + 32],
                                         h1B[LO, cs], dw2t[LO, :], start=True, stop=True)
                        nc.tensor.matmul(xd[:, 32 * c:32 * c + 32],
                                         h1A[HI, cs], dw2t[HI, :], start=True, stop=True)
                        nc.tensor.matmul(xd[:, 128 + 32 * c:128 + 32 * c + 32],
                                         h1B[HI, cs], dw2t[HI, :], start=True, stop=True)

                    # bias adds into merged staging tiles
                    zs = st_pool.tile([128, 1024], F32, tag="zs")
                    nc.vector.tensor_tensor(zs[:, 0:512], ztmA[:, :], zzd_rep[:, :], op=ALU.add)
                    nc.vector.tensor_tensor(zs[:, 512:1024], ztmB[:, :], zzd_rep[:, :], op=ALU.add)
                    xs = st_pool.tile([128, 512], F32, tag="xs")
                    nc.vector.tensor_tensor(xs[:, 0:256], xr[:, 0:256], b2d_rep[:, 0:256], op=ALU.add)
                    nc.vector.tensor_tensor(xs[:, 256:512], xd[:, 0:256], b2d_rep[:, 0:256], op=ALU.add)

                    # keep z[:, -1, :] column for the rollout
                    nc.vector.tensor_copy(zlast[:, j:j + 1], zT[:, S - 1:S])

                    # ---- stores (A+B merged per output) ----
                    # zs free layout: (q: batch A/B, c: 4, w: z|zd, l: 64)
                    zview = zs[:, :].rearrange("p (q c w l) -> p q c w l", q=2, c=4, w=2)
                    nc.sync.dma_start(
                        z_d[bA:bA + 2].rearrange("q (c p) l -> p q c l", p=128),
                        zview[:, :, :, 0, :])
                    nc.scalar.dma_start(
                        zdyn_d[bA:bA + 2].rearrange("q (c p) l -> p q c l", p=128),
                        zview[:, :, :, 1, :])
                    # xs free layout: (o: rec|dyn, q: A/B, c: 4, d: 32)
                    xview = xs[:, :].rearrange("p (o q c d) -> p o q c d", o=2, q=2, c=4)
                    nc.sync.dma_start(
                        xrec_d[bA:bA + 2].rearrange("q (c p) d -> p q c d", p=128),
                        xview[:, 0])
                    nc.scalar.dma_start(
                        xdyn_d[bA:bA + 2].rearrange("q (c p) d -> p q c d", p=128),
                        xview[:, 1])

            # ---------------- prediction rollout ----------------
            with tc.tile_pool(name="pwork", bufs=3) as pw_pool, \
                 tc.tile_pool(name="pmm", bufs=3, space="PSUM") as pmm_pool, \
                 tc.tile_pool(name="pacc", bufs=2, space="PSUM") as pacc_pool:
                xpr_v = xprd_d.rearrange("(b2 two) t d -> b2 two t d", two=2)
                for tb in range(NTB if do_pred else 0):
                    xpp = pacc_pool.tile([128, 512], F32, tag="xpp")
                    if npair < 64:
                        nc.vector.memset(xpp[:, :], 0.0)
                    for sb in range(2):
                        t0 = 16 * tb + 8 * sb  # powers t0+1 .. t0+8
                        zpp = pmm_pool.tile([128, 512], F32, tag="pmm")
                        if npair < 64:
                            nc.vector.memset(zpp[:, :], 0.0)
                        for k in range(8):
                            ks = slice(64 * (t0 + k), 64 * (t0 + k) + 64)
                            nc.tensor.matmul(zpp[LO, 64 * k:64 * k + npair],
                                             kpow[LO, ks], zlast[LO, 0:npair],
                                             start=True, stop=True)
                            nc.tensor.matmul(zpp[HI, 64 * k:64 * k + npair],
                                             kpow[HI, ks], zlast[HI, 0:npair],
                                             start=True, stop=True)
                        zps = pw_pool.tile([128, 512], F32, tag="zps")
                        nc.vector.tensor_copy(zps[:, :], zpp[:, :])
                        h1pp = pmm_pool.tile([128, 512], F32, tag="pmm")
                        nc.tensor.matmul(h1pp[LO, :], dw1t[LO, :], zps[LO, :],
                                         start=True, stop=True)
                        nc.tensor.matmul(h1pp[HI, :], dw1t[HI, :], zps[HI, :],
                                         start=True, stop=True)
                        h1ps = pw_pool.tile([128, 512], F32, tag="h1ps")
                        nc.scalar.activation(h1ps[:, :], h1pp[:, :], AF.Relu, bias=b1d[:, 0:1])
                        for k in range(8):
                            o = 32 * (8 * sb + k)
                            nc.tensor.matmul(xpp[0:npair, o:o + 32],
                                             h1ps[LO, 64 * k:64 * k + npair],
                                             dw2t[LO, :], start=True, stop=True)
                            nc.tensor.matmul(xpp[64:64 + npair, o:o + 32],
                                             h1ps[HI, 64 * k:64 * k + npair],
                                             dw2t[HI, :], start=True, stop=True)
                    xps = pw_pool.tile([128, 512], F32, tag="xps")
                    nc.vector.tensor_tensor(xps[:, :], xpp[:, :], b2d_rep[:, :], op=ALU.add)
                    nc.sync.dma_start(
                        xpr_v[:, 0, 16 * tb:16 * tb + 16, :],
                        xps[0:npair, :].rearrange("p (t d) -> p t d", t=16))
                    nc.scalar.dma_start(
                        xpr_v[:, 1, 16 * tb:16 * tb + 16, :],
                        xps[64:64 + npair, :].rearrange("p (t d) -> p t d", t=16))

    nc.compile()
    return nc


def _prep_xt(x_shard):
    # [128, 512, 32] -> [64 pairs, 64, 512]; rows 0:32 even batch x^T, 32:64 odd
    xt = np.empty((NPAIR, 64, S), np.float32)
    xt[:, 0:32, :] = x_shard[0::2].transpose(0, 2, 1)
    xt[:, 32:64, :] = x_shard[1::2].transpose(0, 2, 1)
    return xt


def _run(inputs, trace=False):
    from concourse import bass_utils

    x = np.asarray(inputs["x"], np.float32)
    P = int(inputs["pred_len"])
    key = P
    if key not in _CACHE:
        _CACHE[key] = _build(P)
    nc = _CACHE[key]

    weights = {k: np.ascontiguousarray(np.asarray(inputs[k], np.float32))
               for k in ("enc_w1", "enc_b1", "enc_w2", "enc_b2",
                         "dec_w1", "dec_b1", "dec_w2", "dec_b2", "K_w")}
    in_maps = []
    for c in range(NCORES):
        m = dict(weights)
        m["xt"] = _prep_xt(x[c * BS:(c + 1) * BS])
        in_maps.append(m)

    res = bass_utils.run_bass_kernel_spmd(nc, in_maps, core_ids=list(range(NCORES)),
                                          trace=trace)
    rs = res.results
    x_rec = np.concatenate([r["x_rec"] for r in rs], 0)
    x_dyn = np.concatenate([r["x_dyn"] for r in rs], 0)
    x_pred = np.concatenate([r["x_pred"] for r in rs], 0)
    z = np.concatenate([r["z"] for r in rs], 0)
    z_dyn = np.concatenate([r["z_dyn"] for r in rs], 0)
    return (x_rec, x_dyn, x_pred, z, z_dyn), res


def kernel(**inputs):
    return _run(inputs)[0]


# revision 19
# speedup vs baseline: 1.2083x; 1.2083x over previous
"""DeepKoopman Trainium2 kernel (8-core data-parallel).

Per core (128-batch shard): activations ride the free dim in "T-form"
(features on partitions); each 512-token chunk is one batch row (S=512).
Batch pairs (A, B) are stacked on partition halves of [128, 512] tiles.

Matmuls use float32r (TF32-like, ~2e-4 rel err, 4x the fp32 PE rate at
N>=256). HW rules discovered on the way (violations lock up the device or
fail codegen):
  - f32r matmul outputs MUST start at PSUM partition 0 (input row strips
    are free) -> batch-pair matmuls use BLOCK-DIAGONAL weights (K=128,
    M=128, dst 0:128) instead of two half-matmuls.
  - Matmuls that can run concurrently (disjoint PE row strips) must write
    DIFFERENT PSUM BANKS - concurrent same-bank drains are fatal.
  - PSUM pool slots are not bank-aligned; tiles are explicitly bank-sized.
Token-major outputs come from "flip" matmuls (lhsT = activationT slice).
z and z_dyn share one flip via rhs = [enc_w2^T | enc_w2^T K_w^T] with a
zero pad to N=256 so f32r runs at full rate; bias [enc_b2 | enc_b2 K_w^T]
is added by DVE. The two decoder hidden layers fuse z_dyn away entirely
via W = dec_w1 @ K_w. The P-step rollout uses K-power matrices
KPOW[:, 64(t-1):64t] = (K^T)^t built by doubling (lo half, then one
SBUF->SBUF DMA replicates to the hi half); x_pred staging is chunk-major
(one batch per partition -> 2KB-contiguous DRAM runs).
"""

import numpy as np

B, S, D, L, H, NCORES = 1024, 512, 32, 64, 64, 8
BS = B // NCORES          # 128 batches per core
NPAIR = BS // 2           # 64 pairs

_CACHE = {}


def _build(P, npair=NPAIR, do_main=True, do_pred=True):
    import concourse.mybir as mybir
    import concourse.tile as tile
    from concourse import bacc

    F32 = mybir.dt.float32
    F32R = mybir.dt.float32r
    AF = mybir.ActivationFunctionType
    ALU = mybir.AluOpType

    assert P % 16 == 0
    NTB = P // 16  # pred time blocks

    nc = bacc.Bacc("TRN2", target_bir_lowering=False, debug=False,
                   enable_asserts=True, num_devices=NCORES)

    xt_d = nc.dram_tensor("xt", [npair, 64, S], F32R, kind="ExternalInput").ap()
    ew1_d = nc.dram_tensor("enc_w1", [H, D], F32, kind="ExternalInput").ap()
    eb1_d = nc.dram_tensor("enc_b1", [H], F32, kind="ExternalInput").ap()
    ew2_d = nc.dram_tensor("enc_w2", [L, H], F32, kind="ExternalInput").ap()
    eb2_d = nc.dram_tensor("enc_b2", [L], F32, kind="ExternalInput").ap()
    dw1_d = nc.dram_tensor("dec_w1", [H, L], F32, kind="ExternalInput").ap()
    db1_d = nc.dram_tensor("dec_b1", [H], F32, kind="ExternalInput").ap()
    dw2_d = nc.dram_tensor("dec_w2", [D, H], F32, kind="ExternalInput").ap()
    db2_d = nc.dram_tensor("dec_b2", [D], F32, kind="ExternalInput").ap()
    kw_d = nc.dram_tensor("K_w", [L, L], F32, kind="ExternalInput").ap()

    xrec_d = nc.dram_tensor("x_rec", [2 * npair, S, D], F32, kind="ExternalOutput").ap()
    xdyn_d = nc.dram_tensor("x_dyn", [2 * npair, S, D], F32, kind="ExternalOutput").ap()
    xprd_d = nc.dram_tensor("x_pred", [2 * npair, P, D], F32, kind="ExternalOutput").ap()
    z_d = nc.dram_tensor("z", [2 * npair, S, L], F32, kind="ExternalOutput").ap()
    zdyn_d = nc.dram_tensor("z_dyn", [2 * npair, S, L], F32, kind="ExternalOutput").ap()

    LO, HI = slice(0, 64), slice(64, 128)

    def r32(ap):
        return ap.bitcast(F32R)

    with tile.TileContext(nc) as tc:
        with tc.tile_pool(name="const", bufs=1) as cp:
            # ---------------- weights / biases / constants ----------------
            # block-diagonal pair weights: [0:64,0:64]=W, [64:128,64:128]=W
            eblk = cp.tile([128, 128], F32R)   # enc_w1^T blocks at rows 0:32/32:64
            ew2blk = cp.tile([128, 128], F32R)  # enc_w2^T block-diag
            dwblk = cp.tile([128, 128], F32R)  # dec_w1^T block-diag
            wfblk = cp.tile([128, 128], F32R)  # (dec_w1 K_w)^T block-diag
            nc.gpsimd.memset(eblk[:, :].bitcast(F32), 0.0)
            nc.gpsimd.memset(ew2blk[:, :].bitcast(F32), 0.0)
            nc.gpsimd.memset(dwblk[:, :].bitcast(F32), 0.0)
            nc.gpsimd.memset(wfblk[:, :].bitcast(F32), 0.0)
            nc.sync.dma_start(eblk[0:32, 0:64], r32(ew1_d.rearrange("a b -> b a")))
            nc.sync.dma_start(eblk[32:64, 64:128], r32(ew1_d.rearrange("a b -> b a")))
            nc.sync.dma_start(ew2blk[LO, 0:64], r32(ew2_d.rearrange("a b -> b a")))
            nc.sync.dma_start(ew2blk[HI, 64:128], r32(ew2_d.rearrange("a b -> b a")))
            nc.sync.dma_start(dwblk[LO, 0:64], r32(dw1_d.rearrange("a b -> b a")))
            nc.sync.dma_start(dwblk[HI, 64:128], r32(dw1_d.rearrange("a b -> b a")))

            ew2s = cp.tile([128, H], F32R)    # enc_w2 straight (lo)
            dw1t = cp.tile([128, H], F32R)    # dec_w1^T lo/hi (pred + wf mm)
            dw2t = cp.tile([128, D], F32R)    # dec_w2^T lo/hi (flips)
            kwt = cp.tile([128, L], F32R)     # K_w^T lo
            kst = cp.tile([128, L], F32R)     # K_w straight lo
            nc.sync.dma_start(ew2s[0:64, :], r32(ew2_d[:, :]))
            nc.sync.dma_start(dw1t[LO, :], r32(dw1_d.rearrange("a b -> b a")))
            nc.sync.dma_start(dw1t[HI, :], r32(dw1_d.rearrange("a b -> b a")))
            nc.sync.dma_start(dw2t[LO, :], r32(dw2_d.rearrange("a b -> b a")))
            nc.sync.dma_start(dw2t[HI, :], r32(dw2_d.rearrange("a b -> b a")))
            nc.sync.dma_start(kwt[LO, :], r32(kw_d.rearrange("a b -> b a")))
            nc.sync.dma_start(kst[LO, :], r32(kw_d[:, :]))

            b1e = cp.tile([128, 1], F32)     # enc_b1 lo/hi (per-partition)
            b2e = cp.tile([128, 1], F32)     # enc_b2 lo/hi
            b1d = cp.tile([128, 1], F32)     # dec_b1 lo/hi
            for t_, src in ((b1e, eb1_d), (b2e, eb2_d), (b1d, db1_d)):
                nc.sync.dma_start(t_[LO, 0:1], src.rearrange("(a b) -> a b", b=1))
                nc.sync.dma_start(t_[HI, 0:1], src.rearrange("(a b) -> a b", b=1))
            b2er = cp.tile([128, 1], F32R)
            nc.vector.tensor_copy(b2er[LO, 0:1], b2e[LO, 0:1])

            # z/z_dyn flip rhs [enc_w2^T | EK | zero pad], lo+hi rows
            zzdw = cp.tile([128, 256], F32R)
            nc.gpsimd.memset(zzdw[:, :].bitcast(F32), 0.0)
            nc.sync.dma_start(zzdw[LO, 0:64], r32(ew2_d.rearrange("a b -> b a")))

            ones = cp.tile([128, 128], F32)
            nc.gpsimd.memset(ones[0:1, :], 1.0)
            zzd_row = cp.tile([128, 512], F32)  # 4x [enc_b2 | enc_b2 @ K_w^T]
            b2d_row = cp.tile([128, 512], F32)  # 16x dec_b2
            for r in range(4):
                nc.sync.dma_start(zzd_row[0:1, 128 * r:128 * r + 64],
                                  eb2_d.rearrange("(a b) -> a b", a=1))
            for r in range(16):
                nc.sync.dma_start(b2d_row[0:1, 32 * r:32 * r + 32],
                                  db2_d.rearrange("(a b) -> a b", a=1))
            zzd_rep = cp.tile([128, 512], F32)
            b2d_rep = cp.tile([128, 512], F32)
            bzrow = cp.tile([128, 64], F32)

            kpow = cp.tile([128, 64 * P], F32R)   # (K^T)^t at [:, 64(t-1):64t]
            zlast = cp.tile([128, L], F32R)       # col j = z[:,-1,:] of pair j
            qcur = cp.tile([128, L], F32R)        # K^m (doubling helper)
            qnxt = cp.tile([128, L], F32R)

            with tc.tile_pool(name="sps", bufs=2, space="PSUM") as sps:
                # EK (lo): EK[h,l'] = sum_l enc_w2[l,h] K^T[l,l']
                ekp = sps.tile([128, 512], F32, tag="s")
                nc.tensor.matmul(ekp[0:64, 0:64], ew2s[0:64, :], kwt[0:64, :],
                                 start=True, stop=True)
                nc.scalar.copy(zzdw[LO, 64:128], ekp[0:64, 0:64])
                # replicate zzdw lo rows -> hi rows (incl. zero pad)
                nc.sync.dma_start(zzdw[HI, :], zzdw[LO, :])

                # wf (lo): (dec_w1 K_w)^T = K_w^T dec_w1^T
                wfp = sps.tile([128, 512], F32, tag="s")
                nc.tensor.matmul(wfp[0:64, 0:64], kst[LO, :], dw1t[LO, :],
                                 start=True, stop=True)
                nc.scalar.copy(wfblk[LO, 0:64], wfp[0:64, 0:64])
                nc.sync.dma_start(wfblk[HI, 64:128], wfblk[LO, 0:64])

                # bzd row = enc_b2 @ K_w^T
                bzp = sps.tile([128, 512], F32, tag="s")
                nc.tensor.matmul(bzp[0:1, 0:64], b2er[LO, 0:1], kwt[LO, :],
                                 start=True, stop=True)
                nc.vector.tensor_copy(bzrow[0:1, :], bzp[0:1, 0:64])
                for r in range(4):
                    nc.vector.tensor_copy(zzd_row[0:1, 128 * r + 64:128 * r + 128],
                                          bzrow[0:1, :])

                # broadcast bias rows to all partitions via K=1 ones matmuls
                bp = sps.tile([128, 512], F32, tag="s")
                nc.tensor.matmul(bp[:, :], ones[0:1, 0:128], zzd_row[0:1, :],
                                 start=True, stop=True)
                nc.vector.tensor_copy(zzd_rep[:, :], bp[:, :])
                bp2 = sps.tile([128, 512], F32, tag="s")
                nc.tensor.matmul(bp2[:, :], ones[0:1, 0:128], b2d_row[0:1, :],
                                 start=True, stop=True)
                nc.vector.tensor_copy(b2d_rep[:, :], bp2[:, :])

                # ---------------- K powers (lo half only) ----------------
                nc.vector.tensor_copy(kpow[LO, 0:64], kwt[LO, :])  # P_1
                for t in range(1, 8):  # P_2..P_8 (serial)
                    pp = sps.tile([128, 512], F32, tag="s")
                    nc.tensor.matmul(pp[0:64, 0:64], kst[LO, :],
                                     kpow[LO, 64 * (t - 1):64 * t],
                                     start=True, stop=True)
                    nc.vector.tensor_copy(kpow[LO, 64 * t:64 * t + 64], pp[0:64, 0:64])
                nc.vector.tensor_copy(qcur[LO, :], kst[LO, :])  # Q_1
                for m in (1, 2, 4):   # Q_2, Q_4, Q_8
                    qp = sps.tile([128, 512], F32, tag="s")
                    nc.tensor.matmul(qp[0:64, 0:64],
                                     kpow[LO, 64 * (m - 1):64 * m],
                                     qcur[LO, :], start=True, stop=True)
                    nc.vector.tensor_copy(qnxt[LO, :], qp[0:64, 0:64])
                    qcur, qnxt = qnxt, qcur
                m = 8
                while m < P:  # S_{m+1..2m} = mm(lhsT=Q_m, rhs=S_{1..m})
                    for ch in range(max(1, (64 * m) // 512)):
                        n0 = 512 * ch if 64 * m > 512 else 0
                        nn = min(512, 64 * m)
                        dp = sps.tile([128, 512], F32, tag="s")
                        nc.tensor.matmul(dp[0:64, 0:nn], qcur[LO, :],
                                         kpow[LO, n0:n0 + nn],
                                         start=True, stop=True)
                        nc.vector.tensor_copy(kpow[LO, 64 * m + n0:64 * m + n0 + nn],
                                              dp[0:64, 0:nn])
                    if 2 * m < P:
                        qp = sps.tile([128, 512], F32, tag="s")
                        nc.tensor.matmul(qp[0:64, 0:64],
                                         kpow[LO, 64 * (m - 1):64 * m],
                                         qcur[LO, :], start=True, stop=True)
                        nc.vector.tensor_copy(qnxt[LO, :], qp[0:64, 0:64])
                        qcur, qnxt = qnxt, qcur
                    m *= 2
                # replicate powers to hi rows for the odd-batch rollout chain
                nc.sync.dma_start(kpow[HI, :], kpow[LO, :])

            # ---------------- main loop over batch pairs ----------------
            # PSUM budget (8 banks): mm x3, ztmA, ztmB, xpA, xpB (+1 spare)
            with tc.tile_pool(name="work", bufs=2) as wp_pool, \
                 tc.tile_pool(name="xin", bufs=3) as xin_pool, \
                 tc.tile_pool(name="stage", bufs=3) as st_pool, \
                 tc.tile_pool(name="mmp", bufs=3, space="PSUM") as mm_pool, \
                 tc.tile_pool(name="tmp", bufs=1, space="PSUM") as tm_pool:

                for j in range(npair if do_main else 0):
                    bA = 2 * j
                    xT = xin_pool.tile([128, S], F32R, tag="xT")
                    nc.sync.dma_start(xT[0:64, :], xt_d[j])

                    # encoder layer 1: block-diag, rows 0:64 -> out pair-stacked
                    pre1 = mm_pool.tile([128, S], F32, tag="mm")
                    nc.tensor.matmul(pre1[:, :], eblk[0:64, :], xT[0:64, :],
                                     start=True, stop=True)
                    h1T = wp_pool.tile([128, S], F32R, tag="h1T")
                    nc.scalar.activation(h1T[:, :], pre1[:, :], AF.Relu, bias=b1e[:, 0:1])

                    # encoder layer 2 (K=128 block-diag)
                    zp = mm_pool.tile([128, S], F32, tag="mm")
                    nc.tensor.matmul(zp[:, :], ew2blk[:, :], h1T[:, :], start=True, stop=True)
                    zT = wp_pool.tile([128, S], F32R, tag="zT")
                    nc.scalar.activation(zT[:, :], zp[:, :], AF.Identity, bias=b2e[:, 0:1])

                    # decoder hidden on z and (fused) on z_dyn, pair-stacked
                    hz = mm_pool.tile([128, S], F32, tag="mm")
                    nc.tensor.matmul(hz[:, :], dwblk[:, :], zT[:, :], start=True, stop=True)
                    h1z = wp_pool.tile([128, S], F32R, tag="h1z")
                    nc.scalar.activation(h1z[:, :], hz[:, :], AF.Relu, bias=b1d[:, 0:1])
                    hd = mm_pool.tile([128, S], F32, tag="mm")
                    nc.tensor.matmul(hd[:, :], wfblk[:, :], zT[:, :], start=True, stop=True)
                    h1d = wp_pool.tile([128, S], F32R, tag="h1d")
                    nc.scalar.activation(h1d[:, :], hd[:, :], AF.Relu, bias=b1d[:, 0:1])

                    # ---- flip-mms (token-major, all dst partition 0) ----
                    # z/zdyn merged; chunks 0..2 zero-padded to N=256 (f32r full rate)
                    ztmA = tm_pool.tile([128, 512], F32, tag="ztmA")
                    ztmB = tm_pool.tile([128, 512], F32, tag="ztmB")
                    for c in range(4):
                        cs = slice(128 * c, 128 * c + 128)
                        nn = 256 if c < 3 else 128
                        nc.tensor.matmul(ztmA[:, 128 * c:128 * c + nn],
                                         h1T[LO, cs], zzdw[LO, 0:nn], start=True, stop=True)
                        nc.tensor.matmul(ztmB[:, 128 * c:128 * c + nn],
                                         h1T[HI, cs], zzdw[HI, 0:nn], start=True, stop=True)
                    # x_rec/x_dyn flips: bank per batch (A: rows 0:64, B: rows 64:128)
                    xpA = tm_pool.tile([128, 512], F32, tag="xpA")  # use [:, 0:256]
                    xpB = tm_pool.tile([128, 512], F32, tag="xpB")
                    for c in range(4):
                        cs = slice(128 * c, 128 * c + 128)
                        nc.tensor.matmul(xpA[:, 32 * c:32 * c + 32],
                                         h1z[LO, cs], dw2t[LO, :], start=True, stop=True)
                        nc.tensor.matmul(xpA[:, 128 + 32 * c:128 + 32 * c + 32],
                                         h1d[LO, cs], dw2t[LO, :], start=True, stop=True)
                        nc.tensor.matmul(xpB[:, 32 * c:32 * c + 32],
                                         h1z[HI, cs], dw2t[HI, :], start=True, stop=True)
                        nc.tensor.matmul(xpB[:, 128 + 32 * c:128 + 32 * c + 32],
                                         h1d[HI, cs], dw2t[HI, :], start=True, stop=True)

                    # bias adds into staging tiles
                    zs = st_pool.tile([128, 1024], F32, tag="zs")
                    nc.vector.tensor_tensor(zs[:, 0:512], ztmA[:, :], zzd_rep[:, :], op=ALU.add)
                    nc.vector.tensor_tensor(zs[:, 512:1024], ztmB[:, :], zzd_rep[:, :], op=ALU.add)
                    xsA = st_pool.tile([128, 256], F32, tag="xsA")
                    xsB = st_pool.tile([128, 256], F32, tag="xsB")
                    nc.vector.tensor_tensor(xsA[:, :], xpA[:, 0:256], b2d_rep[:, 0:256], op=ALU.add)
                    nc.vector.tensor_tensor(xsB[:, :], xpB[:, 0:256], b2d_rep[:, 0:256], op=ALU.add)

                    # keep z[:, -1, :] column for the rollout
                    nc.vector.tensor_copy(zlast[:, j:j + 1], zT[:, S - 1:S])

                    # ---- stores ----
                    # zs free layout: (q: batch A/B, c: 4, w: z|zd, l: 64)
                    zview = zs[:, :].rearrange("p (q c w l) -> p q c w l", q=2, c=4, w=2)
                    nc.sync.dma_start(
                        z_d[bA:bA + 2].rearrange("q (c p) l -> p q c l", p=128),
                        zview[:, :, :, 0, :])
                    nc.scalar.dma_start(
                        zdyn_d[bA:bA + 2].rearrange("q (c p) l -> p q c l", p=128),
                        zview[:, :, :, 1, :])
                    # xs free layout: (o: rec|dyn, c: 4, d: 32) per batch tile
                    for b_, xs_ in ((bA, xsA), (bA + 1, xsB)):
                        xv = xs_[:, :].rearrange("p (o c d) -> p o c d", o=2, c=4)
                        nc.sync.dma_start(
                            xrec_d[b_].rearrange("(c p) d -> p c d", p=128), xv[:, 0])
                        nc.scalar.dma_start(
                            xdyn_d[b_].rearrange("(c p) d -> p c d", p=128), xv[:, 1])

            # ---------------- prediction rollout ----------------
            # chain A = even batches (zlast rows 0:64, kpow lo),
            # chain B = odd batches (rows 64:128, kpow hi); all dst partition 0.
            with tc.tile_pool(name="pwork", bufs=2) as pw_pool, \
                 tc.tile_pool(name="pmm", bufs=1, space="PSUM") as pmm_pool, \
                 tc.tile_pool(name="ph", bufs=1, space="PSUM") as ph_pool, \
                 tc.tile_pool(name="pacc", bufs=2, space="PSUM") as pacc_pool:
                xpr_v = xprd_d.rearrange("(b2 two) t d -> b2 two t d", two=2)
                npq = npair  # pair count = flip M
                for tb in range(NTB if do_pred else 0):
                    xppA = pacc_pool.tile([128, 512], F32, tag="xppA")
                    xppB = pacc_pool.tile([128, 512], F32, tag="xppB")
                    if npair < 64:
                        nc.vector.memset(xppA[:, :], 0.0)
                        nc.vector.memset(xppB[:, :], 0.0)
                    for sb in range(2):
                        t0 = 16 * tb + 8 * sb  # powers t0+1 .. t0+8
                        # zpred for 8 steps: 4 t-pair matmuls (M=128) per chain
                        zppA = pmm_pool.tile([128, 512], F32, tag="zppA")
                        zppB = pmm_pool.tile([128, 512], F32, tag="zppB")
                        if npair < 64:
                            nc.vector.memset(zppA[:, :], 0.0)
                            nc.vector.memset(zppB[:, :], 0.0)
                        for q in range(4):
                            ks = slice(64 * (t0 + 2 * q), 64 * (t0 + 2 * q) + 128)
                            nc.tensor.matmul(zppA[:, 64 * q:64 * q + npq],
                                             kpow[LO, ks], zlast[LO, 0:npq],
                                             start=True, stop=True)
                            nc.tensor.matmul(zppB[:, 64 * q:64 * q + npq],
                                             kpow[HI, ks], zlast[HI, 0:npq],
                                             start=True, stop=True)
                        # partitions of zpp: 0:64 = t-even l, 64:128 = t-odd l
                        zpsA = pw_pool.tile([128, 256], F32R, tag="zpsA")
                        zpsB = pw_pool.tile([128, 256], F32R, tag="zpsB")
                        nc.vector.tensor_copy(zpsA[:, :], zppA[:, 0:256])
                        nc.vector.tensor_copy(zpsB[:, :], zppB[:, 0:256])
                        # decoder hidden: parity e reads rows 0:64, parity o rows 64:128
                        hE = ph_pool.tile([128, 512], F32, tag="hE")
                        hO = ph_pool.tile([128, 512], F32, tag="hO")
                        nc.tensor.matmul(hE[0:64, 0:256], dw1t[LO, :], zpsA[LO, :],
                                         start=True, stop=True)
                        nc.tensor.matmul(hO[0:64, 0:256], dw1t[HI, :], zpsA[HI, :],
                                         start=True, stop=True)
                        nc.tensor.matmul(hE[0:64, 256:512], dw1t[LO, :], zpsB[LO, :],
                                         start=True, stop=True)
                        nc.tensor.matmul(hO[0:64, 256:512], dw1t[HI, :], zpsB[HI, :],
                                         start=True, stop=True)
                        h1E = pw_pool.tile([128, 512], F32R, tag="h1E")
                        h1O = pw_pool.tile([128, 512], F32R, tag="h1O")
                        nc.scalar.activation(h1E[0:64, :], hE[0:64, :], AF.Relu, bias=b1d[LO, 0:1])
                        nc.scalar.activation(h1O[0:64, :], hO[0:64, :], AF.Relu, bias=b1d[LO, 0:1])
                        # flips: t = t0 + 2q + par; chain A cols 0:256, B cols 256:512
                        for q in range(4):
                            for par, h1 in ((0, h1E), (1, h1O)):
                                tloc = 8 * sb + 2 * q + par
                                nc.tensor.matmul(
                                    xppA[0:npq, 32 * tloc:32 * tloc + 32],
                                    h1[0:64, 64 * q:64 * q + npq], dw2t[LO, :],
                                    start=True, stop=True)
                                nc.tensor.matmul(
                                    xppB[0:npq, 32 * tloc:32 * tloc + 32],
                                    h1[0:64, 256 + 64 * q:256 + 64 * q + npq], dw2t[LO, :],
                                    start=True, stop=True)
                    xpsA = pw_pool.tile([128, 512], F32, tag="xpsA")
                    xpsB = pw_pool.tile([128, 512], F32, tag="xpsB")
                    nc.vector.tensor_tensor(xpsA[0:64, :], xppA[0:64, :], b2d_rep[0:64, :], op=ALU.add)
                    nc.vector.tensor_tensor(xpsB[0:64, :], xppB[0:64, :], b2d_rep[0:64, :], op=ALU.add)
                    nc.sync.dma_start(
                        xpr_v[:, 0, 16 * tb:16 * tb + 16, :],
                        xpsA[0:npq, :].rearrange("p (t d) -> p t d", t=16))
                    nc.scalar.dma_start(
                        xpr_v[:, 1, 16 * tb:16 * tb + 16, :],
                        xpsB[0:npq, :].rearrange("p (t d) -> p t d", t=16))

    nc.compile()
    return nc


def _prep_xt(x_shard):
    # [128, 512, 32] -> [64 pairs, 64, 512]; rows 0:32 even batch x^T, 32:64 odd
    xt = np.empty((NPAIR, 64, S), np.float32)
    xt[:, 0:32, :] = x_shard[0::2].transpose(0, 2, 1)
    xt[:, 32:64, :] = x_shard[1::2].transpose(0, 2, 1)
    return xt


def _run(inputs, trace=False):
    from concourse import bass_utils

    x = np.asarray(inputs["x"], np.float32)
    P = int(inputs["pred_len"])
    key = P
    if key not in _CACHE:
        _CACHE[key] = _build(P)
    nc = _CACHE[key]

    weights = {k: np.ascontiguousarray(np.asarray(inputs[k], np.float32))
               for k in ("enc_w1", "enc_b1", "enc_w2", "enc_b2",
                         "dec_w1", "dec_b1", "dec_w2", "dec_b2", "K_w")}
    in_maps = []
    for c in range(NCORES):
        m = dict(weights)
        m["xt"] = _prep_xt(x[c * BS:(c + 1) * BS])
        in_maps.append(m)

    res = bass_utils.run_bass_kernel_spmd(nc, in_maps, core_ids=list(range(NCORES)),
                                          trace=trace)
    rs = res.results
    x_rec = np.concatenate([r["x_rec"] for r in rs], 0)
    x_dyn = np.concatenate([r["x_dyn"] for r in rs], 0)
    x_pred = np.concatenate([r["x_pred"] for r in rs], 0)
    z = np.concatenate([r["z"] for r in rs], 0)
    z_dyn = np.concatenate([r["z_dyn"] for r in rs], 0)
    return (x_rec, x_dyn, x_pred, z, z_dyn), res


def kernel(**inputs):
    return _run(inputs)[0]


# revision 21
# speedup vs baseline: 1.8297x; 1.5143x over previous
"""DeepKoopman Trainium2 kernel (8-core data-parallel).

Per core (128-batch shard): activations ride the free dim in "T-form"
(features on partitions); each 512-token chunk is one batch row (S=512).
Batch pairs (A, B) are stacked on partition halves of [128, 512] tiles.

Matmuls use float32r (TF32-like, ~2e-4 rel err, 4x the fp32 PE rate at
N>=256). HW rules discovered on the way (violations lock up the device or
fail codegen):
  - f32r matmul outputs MUST start at PSUM partition 0 (input row strips
    are free) -> batch-pair matmuls use BLOCK-DIAGONAL weights (K=128,
    M=128, dst 0:128) instead of two half-matmuls.
  - Matmuls that can run concurrently (disjoint PE row strips) must write
    DIFFERENT PSUM BANKS - concurrent same-bank drains are fatal.
  - PSUM pool slots are not bank-aligned; tiles are explicitly bank-sized.
Token-major outputs come from "flip" matmuls (lhsT = activationT slice).
z and z_dyn share one flip via rhs = [enc_w2^T | enc_w2^T K_w^T] with a
zero pad to N=256 so f32r runs at full rate; bias [enc_b2 | enc_b2 K_w^T]
is added by DVE. The two decoder hidden layers fuse z_dyn away entirely
via W = dec_w1 @ K_w. The P-step rollout uses K-power matrices
KPOW[:, 64(t-1):64t] = (K^T)^t built by doubling (lo half, then one
SBUF->SBUF DMA replicates to the hi half); x_pred staging is chunk-major
(one batch per partition -> 2KB-contiguous DRAM runs).
"""

import numpy as np

B, S, D, L, H, NCORES = 1024, 512, 32, 64, 64, 8
BS = B // NCORES          # 128 batches per core
NPAIR = BS // 2           # 64 pairs

_CACHE = {}


def _build(P, npair=NPAIR, do_main=True, do_pred=True):
    import concourse.mybir as mybir
    import concourse.tile as tile
    from concourse import bacc

    F32 = mybir.dt.float32
    F32R = mybir.dt.float32r
    AF = mybir.ActivationFunctionType
    ALU = mybir.AluOpType

    assert P % 16 == 0
    NTB = P // 16  # pred time blocks

    nc = bacc.Bacc("TRN2", target_bir_lowering=False, debug=False,
                   enable_asserts=True, num_devices=NCORES)

    xt_d = nc.dram_tensor("xt", [npair, 64, S], F32R, kind="ExternalInput").ap()
    ew1_d = nc.dram_tensor("enc_w1", [H, D], F32, kind="ExternalInput").ap()
    eb1_d = nc.dram_tensor("enc_b1", [H], F32, kind="ExternalInput").ap()
    ew2_d = nc.dram_tensor("enc_w2", [L, H], F32, kind="ExternalInput").ap()
    eb2_d = nc.dram_tensor("enc_b2", [L], F32, kind="ExternalInput").ap()
    dw1_d = nc.dram_tensor("dec_w1", [H, L], F32, kind="ExternalInput").ap()
    db1_d = nc.dram_tensor("dec_b1", [H], F32, kind="ExternalInput").ap()
    dw2_d = nc.dram_tensor("dec_w2", [D, H], F32, kind="ExternalInput").ap()
    db2_d = nc.dram_tensor("dec_b2", [D], F32, kind="ExternalInput").ap()
    kw_d = nc.dram_tensor("K_w", [L, L], F32, kind="ExternalInput").ap()

    xrec_d = nc.dram_tensor("x_rec", [2 * npair, S, D], F32, kind="ExternalOutput").ap()
    xdyn_d = nc.dram_tensor("x_dyn", [2 * npair, S, D], F32, kind="ExternalOutput").ap()
    xprd_d = nc.dram_tensor("x_pred", [2 * npair, P, D], F32, kind="ExternalOutput").ap()
    z_d = nc.dram_tensor("z", [2 * npair, S, L], F32, kind="ExternalOutput").ap()
    zdyn_d = nc.dram_tensor("z_dyn", [2 * npair, S, L], F32, kind="ExternalOutput").ap()

    LO, HI = slice(0, 64), slice(64, 128)

    def r32(ap):
        return ap.bitcast(F32R)

    with tile.TileContext(nc) as tc:
        with tc.tile_pool(name="const", bufs=1) as cp:
            # ---------------- weights / biases / constants ----------------
            # block-diagonal pair weights: [0:64,0:64]=W, [64:128,64:128]=W
            eblk = cp.tile([128, 128], F32R)   # enc_w1^T blocks at rows 0:32/32:64
            ew2blk = cp.tile([128, 128], F32R)  # enc_w2^T block-diag
            dwblk = cp.tile([128, 128], F32R)  # dec_w1^T block-diag
            wfblk = cp.tile([128, 128], F32R)  # (dec_w1 K_w)^T block-diag
            nc.gpsimd.memset(eblk[:, :].bitcast(F32), 0.0)
            nc.gpsimd.memset(ew2blk[:, :].bitcast(F32), 0.0)
            nc.gpsimd.memset(dwblk[:, :].bitcast(F32), 0.0)
            nc.gpsimd.memset(wfblk[:, :].bitcast(F32), 0.0)
            nc.sync.dma_start(eblk[0:32, 0:64], r32(ew1_d.rearrange("a b -> b a")))
            nc.sync.dma_start(eblk[32:64, 64:128], r32(ew1_d.rearrange("a b -> b a")))
            nc.sync.dma_start(ew2blk[LO, 0:64], r32(ew2_d.rearrange("a b -> b a")))
            nc.sync.dma_start(ew2blk[HI, 64:128], r32(ew2_d.rearrange("a b -> b a")))
            nc.sync.dma_start(dwblk[LO, 0:64], r32(dw1_d.rearrange("a b -> b a")))
            nc.sync.dma_start(dwblk[HI, 64:128], r32(dw1_d.rearrange("a b -> b a")))

            ew2s = cp.tile([128, H], F32R)    # enc_w2 straight (lo)
            dw1t = cp.tile([128, H], F32R)    # dec_w1^T lo/hi (pred + wf mm)
            dw2t = cp.tile([128, D], F32R)    # dec_w2^T lo/hi (flips)
            kwt = cp.tile([128, L], F32R)     # K_w^T lo
            kst = cp.tile([128, L], F32R)     # K_w straight lo
            nc.sync.dma_start(ew2s[0:64, :], r32(ew2_d[:, :]))
            nc.sync.dma_start(dw1t[LO, :], r32(dw1_d.rearrange("a b -> b a")))
            nc.sync.dma_start(dw1t[HI, :], r32(dw1_d.rearrange("a b -> b a")))
            nc.sync.dma_start(dw2t[LO, :], r32(dw2_d.rearrange("a b -> b a")))
            nc.sync.dma_start(dw2t[HI, :], r32(dw2_d.rearrange("a b -> b a")))
            nc.sync.dma_start(kwt[LO, :], r32(kw_d.rearrange("a b -> b a")))
            nc.sync.dma_start(kst[LO, :], r32(kw_d[:, :]))

            b1e = cp.tile([128, 1], F32)     # enc_b1 lo/hi (per-partition)
            b2e = cp.tile([128, 1], F32)     # enc_b2 lo/hi
            b1d = cp.tile([128, 1], F32)     # dec_b1 lo/hi
            for t_, src in ((b1e, eb1_d), (b2e, eb2_d), (b1d, db1_d)):
                nc.sync.dma_start(t_[LO, 0:1], src.rearrange("(a b) -> a b", b=1))
                nc.sync.dma_start(t_[HI, 0:1], src.rearrange("(a b) -> a b", b=1))
            b2er = cp.tile([128, 1], F32R)
            nc.vector.tensor_copy(b2er[LO, 0:1], b2e[LO, 0:1])

            # z/z_dyn flip rhs [enc_w2^T | EK | zero pad], lo+hi rows
            zzdw = cp.tile([128, 256], F32R)
            nc.gpsimd.memset(zzdw[:, :].bitcast(F32), 0.0)
            nc.sync.dma_start(zzdw[LO, 0:64], r32(ew2_d.rearrange("a b -> b a")))

            ones = cp.tile([128, 128], F32)
            nc.gpsimd.memset(ones[0:1, :], 1.0)
            zzd_row = cp.tile([128, 512], F32)  # 4x [enc_b2 | enc_b2 @ K_w^T]
            b2d_row = cp.tile([128, 512], F32)  # 16x dec_b2
            for r in range(4):
                nc.sync.dma_start(zzd_row[0:1, 128 * r:128 * r + 64],
                                  eb2_d.rearrange("(a b) -> a b", a=1))
            for r in range(16):
                nc.sync.dma_start(b2d_row[0:1, 32 * r:32 * r + 32],
                                  db2_d.rearrange("(a b) -> a b", a=1))
            zzd_rep = cp.tile([128, 512], F32)
            b2d_rep = cp.tile([128, 512], F32)
            bzrow = cp.tile([128, 64], F32)

            kpow = cp.tile([128, 64 * P], F32R)   # (K^T)^t at [:, 64(t-1):64t]
            zlast = cp.tile([128, L], F32R)       # col j = z[:,-1,:] of pair j
            qcur = cp.tile([128, L], F32R)        # K^m (doubling helper)
            qnxt = cp.tile([128, L], F32R)

            with tc.tile_pool(name="sps", bufs=2, space="PSUM") as sps:
                # EK (lo): EK[h,l'] = sum_l enc_w2[l,h] K^T[l,l']
                ekp = sps.tile([128, 512], F32, tag="s")
                nc.tensor.matmul(ekp[0:64, 0:64], ew2s[0:64, :], kwt[0:64, :],
                                 start=True, stop=True)
                nc.scalar.copy(zzdw[LO, 64:128], ekp[0:64, 0:64])
                # replicate zzdw lo rows -> hi rows (incl. zero pad)
                nc.sync.dma_start(zzdw[HI, :], zzdw[LO, :])

                # wf (lo): (dec_w1 K_w)^T = K_w^T dec_w1^T
                wfp = sps.tile([128, 512], F32, tag="s")
                nc.tensor.matmul(wfp[0:64, 0:64], kst[LO, :], dw1t[LO, :],
                                 start=True, stop=True)
                nc.scalar.copy(wfblk[LO, 0:64], wfp[0:64, 0:64])
                nc.sync.dma_start(wfblk[HI, 64:128], wfblk[LO, 0:64])

                # bzd row = enc_b2 @ K_w^T
                bzp = sps.tile([128, 512], F32, tag="s")
                nc.tensor.matmul(bzp[0:1, 0:64], b2er[LO, 0:1], kwt[LO, :],
                                 start=True, stop=True)
                nc.vector.tensor_copy(bzrow[0:1, :], bzp[0:1, 0:64])
                for r in range(4):
                    nc.vector.tensor_copy(zzd_row[0:1, 128 * r + 64:128 * r + 128],
                                          bzrow[0:1, :])

                # broadcast bias rows to all partitions via K=1 ones matmuls
                bp = sps.tile([128, 512], F32, tag="s")
                nc.tensor.matmul(bp[:, :], ones[0:1, 0:128], zzd_row[0:1, :],
                                 start=True, stop=True)
                nc.vector.tensor_copy(zzd_rep[:, :], bp[:, :])
                bp2 = sps.tile([128, 512], F32, tag="s")
                nc.tensor.matmul(bp2[:, :], ones[0:1, 0:128], b2d_row[0:1, :],
                                 start=True, stop=True)
                nc.vector.tensor_copy(b2d_rep[:, :], bp2[:, :])

                # ---------------- K powers (lo half only) ----------------
                nc.vector.tensor_copy(kpow[LO, 0:64], kwt[LO, :])  # P_1
                for t in range(1, 8):  # P_2..P_8 (serial)
                    pp = sps.tile([128, 512], F32, tag="s")
                    nc.tensor.matmul(pp[0:64, 0:64], kst[LO, :],
                                     kpow[LO, 64 * (t - 1):64 * t],
                                     start=True, stop=True)
                    nc.vector.tensor_copy(kpow[LO, 64 * t:64 * t + 64], pp[0:64, 0:64])
                nc.vector.tensor_copy(qcur[LO, :], kst[LO, :])  # Q_1
                for m in (1, 2, 4):   # Q_2, Q_4, Q_8
                    qp = sps.tile([128, 512], F32, tag="s")
                    nc.tensor.matmul(qp[0:64, 0:64],
                                     kpow[LO, 64 * (m - 1):64 * m],
                                     qcur[LO, :], start=True, stop=True)
                    nc.vector.tensor_copy(qnxt[LO, :], qp[0:64, 0:64])
                    qcur, qnxt = qnxt, qcur
                m = 8
                while m < P:  # S_{m+1..2m} = mm(lhsT=Q_m, rhs=S_{1..m})
                    for ch in range(max(1, (64 * m) // 512)):
                        n0 = 512 * ch if 64 * m > 512 else 0
                        nn = min(512, 64 * m)
                        dp = sps.tile([128, 512], F32, tag="s")
                        nc.tensor.matmul(dp[0:64, 0:nn], qcur[LO, :],
                                         kpow[LO, n0:n0 + nn],
                                         start=True, stop=True)
                        nc.vector.tensor_copy(kpow[LO, 64 * m + n0:64 * m + n0 + nn],
                                              dp[0:64, 0:nn])
                    if 2 * m < P:
                        qp = sps.tile([128, 512], F32, tag="s")
                        nc.tensor.matmul(qp[0:64, 0:64],
                                         kpow[LO, 64 * (m - 1):64 * m],
                                         qcur[LO, :], start=True, stop=True)
                        nc.vector.tensor_copy(qnxt[LO, :], qp[0:64, 0:64])
                        qcur, qnxt = qnxt, qcur
                    m *= 2
                # replicate powers to hi rows for the odd-batch rollout chain
                nc.sync.dma_start(kpow[HI, :], kpow[LO, :])

            # ---------------- main loop over batch pairs ----------------
            # PSUM budget (8 banks): mm x3, ztmA, ztmB, xpA, xpB (+1 spare)
            with tc.tile_pool(name="work", bufs=2) as wp_pool, \
                 tc.tile_pool(name="xin", bufs=3) as xin_pool, \
                 tc.tile_pool(name="stage", bufs=3) as st_pool, \
                 tc.tile_pool(name="mmp", bufs=3, space="PSUM") as mm_pool, \
                 tc.tile_pool(name="tmp", bufs=1, space="PSUM") as tm_pool:

                for j in range(npair if do_main else 0):
                    bA = 2 * j
                    xT = xin_pool.tile([128, S], F32R, tag="xT")
                    nc.sync.dma_start(xT[0:64, :], xt_d[j])

                    # encoder layer 1: block-diag, rows 0:64 -> out pair-stacked
                    pre1 = mm_pool.tile([128, S], F32, tag="mm")
                    nc.tensor.matmul(pre1[:, :], eblk[0:64, :], xT[0:64, :],
                                     start=True, stop=True)
                    h1T = wp_pool.tile([128, S], F32R, tag="h1T")
                    nc.scalar.activation(h1T[:, :], pre1[:, :], AF.Relu, bias=b1e[:, 0:1])

                    # encoder layer 2 (K=128 block-diag)
                    zp = mm_pool.tile([128, S], F32, tag="mm")
                    nc.tensor.matmul(zp[:, :], ew2blk[:, :], h1T[:, :], start=True, stop=True)
                    zT = wp_pool.tile([128, S], F32R, tag="zT")
                    nc.scalar.activation(zT[:, :], zp[:, :], AF.Identity, bias=b2e[:, 0:1])

                    # decoder hidden on z and (fused) on z_dyn, pair-stacked
                    hz = mm_pool.tile([128, S], F32, tag="mm")
                    nc.tensor.matmul(hz[:, :], dwblk[:, :], zT[:, :], start=True, stop=True)
                    h1z = wp_pool.tile([128, S], F32R, tag="h1z")
                    nc.scalar.activation(h1z[:, :], hz[:, :], AF.Relu, bias=b1d[:, 0:1])
                    hd = mm_pool.tile([128, S], F32, tag="mm")
                    nc.tensor.matmul(hd[:, :], wfblk[:, :], zT[:, :], start=True, stop=True)
                    h1d = wp_pool.tile([128, S], F32R, tag="h1d")
                    nc.scalar.activation(h1d[:, :], hd[:, :], AF.Relu, bias=b1d[:, 0:1])

                    # ---- flip-mms (token-major, all dst partition 0) ----
                    # z/zdyn merged; chunks 0..2 zero-padded to N=256 (f32r full rate)
                    ztmA = tm_pool.tile([128, 512], F32, tag="ztmA")
                    ztmB = tm_pool.tile([128, 512], F32, tag="ztmB")
                    for c in range(4):
                        cs = slice(128 * c, 128 * c + 128)
                        nn = 256 if c < 3 else 128
                        nc.tensor.matmul(ztmA[:, 128 * c:128 * c + nn],
                                         h1T[LO, cs], zzdw[LO, 0:nn], start=True, stop=True)
                        nc.tensor.matmul(ztmB[:, 128 * c:128 * c + nn],
                                         h1T[HI, cs], zzdw[HI, 0:nn], start=True, stop=True)
                    # x_rec/x_dyn flips: bank per batch (A: rows 0:64, B: rows 64:128)
                    xpA = tm_pool.tile([128, 512], F32, tag="xpA")  # use [:, 0:256]
                    xpB = tm_pool.tile([128, 512], F32, tag="xpB")
                    for c in range(4):
                        cs = slice(128 * c, 128 * c + 128)
                        nc.tensor.matmul(xpA[:, 32 * c:32 * c + 32],
                                         h1z[LO, cs], dw2t[LO, :], start=True, stop=True)
                        nc.tensor.matmul(xpA[:, 128 + 32 * c:128 + 32 * c + 32],
                                         h1d[LO, cs], dw2t[LO, :], start=True, stop=True)
                        nc.tensor.matmul(xpB[:, 32 * c:32 * c + 32],
                                         h1z[HI, cs], dw2t[HI, :], start=True, stop=True)
                        nc.tensor.matmul(xpB[:, 128 + 32 * c:128 + 32 * c + 32],
                                         h1d[HI, cs], dw2t[HI, :], start=True, stop=True)

                    # bias adds into 2-pair staging tiles (stores batched
                    # over 4 batches to halve DMA count). Layouts put the
                    # output-tensor split OUTERMOST so DMA APs stay <=3 dims:
                    # zs2: [z (q c l) | zdyn (q c l)], xs4: [rec (q c d) | dyn ...]
                    if j % 2 == 0:
                        zs2 = st_pool.tile([128, 2048], F32, tag="zs2")
                        xs4 = st_pool.tile([128, 1024], F32, tag="xs4")
                    zv2 = zs2[:, :].rearrange("p (w q c l) -> p w q c l", w=2, q=4, c=4)
                    xv4 = xs4[:, :].rearrange("p (o q c d) -> p o q c d", o=2, q=4, c=4)
                    qA, qB = 2 * (j % 2), 2 * (j % 2) + 1
                    nc.vector.tensor_tensor(
                        zv2[:, :, qA], ztmA[:, :].rearrange("p (c w l) -> p w c l", c=4, w=2),
                        zzd_rep[:, :].rearrange("p (c w l) -> p w c l", c=4, w=2), op=ALU.add)
                    nc.vector.tensor_tensor(
                        zv2[:, :, qB], ztmB[:, :].rearrange("p (c w l) -> p w c l", c=4, w=2),
                        zzd_rep[:, :].rearrange("p (c w l) -> p w c l", c=4, w=2), op=ALU.add)
                    nc.vector.tensor_tensor(
                        xv4[:, :, qA], xpA[:, 0:256].rearrange("p (o cd) -> p o cd", o=2),
                        b2d_rep[:, 0:256].rearrange("p (o cd) -> p o cd", o=2), op=ALU.add)
                    nc.vector.tensor_tensor(
                        xv4[:, :, qB], xpB[:, 0:256].rearrange("p (o cd) -> p o cd", o=2),
                        b2d_rep[:, 0:256].rearrange("p (o cd) -> p o cd", o=2), op=ALU.add)

                    # keep z[:, -1, :] column for the rollout
                    nc.vector.tensor_copy(zlast[:, j:j + 1], zT[:, S - 1:S])

                    # ---- stores (every second pair, 4 batches per DMA) ----
                    if j % 2 == 1 or j == npair - 1:
                        b0 = bA - 2 * (j % 2)
                        nb = 2 * (j % 2) + 2
                        nc.sync.dma_start(
                            z_d[b0:b0 + nb].rearrange("q (c p) l -> p q c l", p=128),
                            zs2[:, 0:256 * nb].rearrange("p (q c l) -> p q c l", q=nb, c=4))
                        nc.scalar.dma_start(
                            zdyn_d[b0:b0 + nb].rearrange("q (c p) l -> p q c l", p=128),
                            zs2[:, 1024:1024 + 256 * nb].rearrange("p (q c l) -> p q c l", q=nb, c=4))
                        nc.sync.dma_start(
                            xrec_d[b0:b0 + nb].rearrange("q (c p) d -> p q c d", p=128),
                            xs4[:, 0:128 * nb].rearrange("p (q c d) -> p q c d", q=nb, c=4))
                        nc.scalar.dma_start(
                            xdyn_d[b0:b0 + nb].rearrange("q (c p) d -> p q c d", p=128),
                            xs4[:, 512:512 + 128 * nb].rearrange("p (q c d) -> p q c d", q=nb, c=4))

            # ---------------- prediction rollout ----------------
            # chain A = even batches (zlast rows 0:64, kpow lo),
            # chain B = odd batches (rows 64:128, kpow hi); all dst partition 0.
            with tc.tile_pool(name="pwork", bufs=2) as pw_pool, \
                 tc.tile_pool(name="pmm", bufs=1, space="PSUM") as pmm_pool, \
                 tc.tile_pool(name="ph", bufs=1, space="PSUM") as ph_pool, \
                 tc.tile_pool(name="pacc", bufs=2, space="PSUM") as pacc_pool:
                xpr_v = xprd_d.rearrange("(b2 two) t d -> b2 two t d", two=2)
                npq = npair  # pair count = flip M
                for tb in range(NTB if do_pred else 0):
                    xppA = pacc_pool.tile([128, 512], F32, tag="xppA")
                    xppB = pacc_pool.tile([128, 512], F32, tag="xppB")
                    if npair < 64:
                        nc.vector.memset(xppA[:, :], 0.0)
                        nc.vector.memset(xppB[:, :], 0.0)
                    for sb in range(2):
                        t0 = 16 * tb + 8 * sb  # powers t0+1 .. t0+8
                        # zpred for 8 steps: 4 t-pair matmuls (M=128) per chain
                        zppA = pmm_pool.tile([128, 512], F32, tag="zppA")
                        zppB = pmm_pool.tile([128, 512], F32, tag="zppB")
                        if npair < 64:
                            nc.vector.memset(zppA[:, :], 0.0)
                            nc.vector.memset(zppB[:, :], 0.0)
                        for q in range(4):
                            ks = slice(64 * (t0 + 2 * q), 64 * (t0 + 2 * q) + 128)
                            nc.tensor.matmul(zppA[:, 64 * q:64 * q + npq],
                                             kpow[LO, ks], zlast[LO, 0:npq],
                                             start=True, stop=True)
                            nc.tensor.matmul(zppB[:, 64 * q:64 * q + npq],
                                             kpow[HI, ks], zlast[HI, 0:npq],
                                             start=True, stop=True)
                        # partitions of zpp: 0:64 = t-even l, 64:128 = t-odd l
                        zpsA = pw_pool.tile([128, 256], F32R, tag="zpsA")
                        zpsB = pw_pool.tile([128, 256], F32R, tag="zpsB")
                        nc.vector.tensor_copy(zpsA[:, :], zppA[:, 0:256])
                        nc.vector.tensor_copy(zpsB[:, :], zppB[:, 0:256])
                        # decoder hidden: parity e reads rows 0:64, parity o rows 64:128
                        hE = ph_pool.tile([128, 512], F32, tag="hE")
                        hO = ph_pool.tile([128, 512], F32, tag="hO")
                        nc.tensor.matmul(hE[0:64, 0:256], dw1t[LO, :], zpsA[LO, :],
                                         start=True, stop=True)
                        nc.tensor.matmul(hO[0:64, 0:256], dw1t[HI, :], zpsA[HI, :],
                                         start=True, stop=True)
                        nc.tensor.matmul(hE[0:64, 256:512], dw1t[LO, :], zpsB[LO, :],
                                         start=True, stop=True)
                        nc.tensor.matmul(hO[0:64, 256:512], dw1t[HI, :], zpsB[HI, :],
                                         start=True, stop=True)
                        h1E = pw_pool.tile([128, 512], F32R, tag="h1E")
                        h1O = pw_pool.tile([128, 512], F32R, tag="h1O")
                        nc.scalar.activation(h1E[0:64, :], hE[0:64, :], AF.Relu, bias=b1d[LO, 0:1])
                        nc.scalar.activation(h1O[0:64, :], hO[0:64, :], AF.Relu, bias=b1d[LO, 0:1])
                        # flips: t = t0 + 2q + par; chain A cols 0:256, B cols 256:512
                        for q in range(4):
                            for par, h1 in ((0, h1E), (1, h1O)):
                                tloc = 8 * sb + 2 * q + par
                                nc.tensor.matmul(
                                    xppA[0:npq, 32 * tloc:32 * tloc + 32],
                                    h1[0:64, 64 * q:64 * q + npq], dw2t[LO, :],
                                    start=True, stop=True)
                                nc.tensor.matmul(
                                    xppB[0:npq, 32 * tloc:32 * tloc + 32],
                                    h1[0:64, 256 + 64 * q:256 + 64 * q + npq], dw2t[LO, :],
                                    start=True, stop=True)
                    xpsA = pw_pool.tile([128, 512], F32, tag="xpsA")
                    xpsB = pw_pool.tile([128, 512], F32, tag="xpsB")
                    nc.vector.tensor_tensor(xpsA[0:64, :], xppA[0:64, :], b2d_rep[0:64, :], op=ALU.add)
                    nc.vector.tensor_tensor(xpsB[0:64, :], xppB[0:64, :], b2d_rep[0:64, :], op=ALU.add)
                    nc.sync.dma_start(
                        xpr_v[:, 0, 16 * tb:16 * tb + 16, :],
                        xpsA[0:npq, :].rearrange("p (t d) -> p t d", t=16))
                    nc.scalar.dma_start(
                        xpr_v[:, 1, 16 * tb:16 * tb + 16, :],
                        xpsB[0:npq, :].rearrange("p (t d) -> p t d", t=16))

    nc.compile()
    return nc


def _prep_xt(x_shard):
    # [128, 512, 32] -> [64 pairs, 64, 512]; rows 0:32 even batch x^T, 32:64 odd
    xt = np.empty((NPAIR, 64, S), np.float32)
    xt[:, 0:32, :] = x_shard[0::2].transpose(0, 2, 1)
    xt[:, 32:64, :] = x_shard[1::2].transpose(0, 2, 1)
    return xt


def _run(inputs, trace=False):
    from concourse import bass_utils

    x = np.asarray(inputs["x"], np.float32)
    P = int(inputs["pred_len"])
    key = P
    if key not in _CACHE:
        _CACHE[key] = _build(P)
    nc = _CACHE[key]

    weights = {k: np.ascontiguousarray(np.asarray(inputs[k], np.float32))
               for k in ("enc_w1", "enc_b1", "enc_w2", "enc_b2",
                         "dec_w1", "dec_b1", "dec_w2", "dec_b2", "K_w")}
    in_maps = []
    for c in range(NCORES):
        m = dict(weights)
        m["xt"] = _prep_xt(x[c * BS:(c + 1) * BS])
        in_maps.append(m)

    res = bass_utils.run_bass_kernel_spmd(nc, in_maps, core_ids=list(range(NCORES)),
                                          trace=trace)
    rs = res.results
    x_rec = np.concatenate([r["x_rec"] for r in rs], 0)
    x_dyn = np.concatenate([r["x_dyn"] for r in rs], 0)
    x_pred = np.concatenate([r["x_pred"] for r in rs], 0)
    z = np.concatenate([r["z"] for r in rs], 0)
    z_dyn = np.concatenate([r["z_dyn"] for r in rs], 0)
    return (x_rec, x_dyn, x_pred, z, z_dyn), res


def kernel(**inputs):
    return _run(inputs)[0]


# revision 25
# speedup vs baseline: 1.9701x; 1.0767x over previous
"""DeepKoopman Trainium2 kernel (8-core data-parallel).

Per core (128-batch shard): activations ride the free dim in "T-form"
(features on partitions); each 512-token chunk is one batch row (S=512).
Batch pairs (A, B) are stacked on partition halves of [128, 512] tiles.

Matmuls use float32r (TF32-like, ~2e-4 rel err, 4x the fp32 PE rate at
N>=256). HW rules discovered on the way (violations lock up the device or
fail codegen):
  - f32r matmul outputs MUST start at PSUM partition 0 (input row strips
    are free) -> batch-pair matmuls use BLOCK-DIAGONAL weights (K=128,
    M=128, dst 0:128) instead of two half-matmuls.
  - Matmuls that can run concurrently (disjoint PE row strips) must write
    DIFFERENT PSUM BANKS - concurrent same-bank drains are fatal.
  - PSUM pool slots are not bank-aligned; tiles are explicitly bank-sized.
Token-major outputs come from "flip" matmuls (lhsT = activationT slice).
z and z_dyn share one flip via rhs = [enc_w2^T | enc_w2^T K_w^T] with a
zero pad to N=256 so f32r runs at full rate; bias [enc_b2 | enc_b2 K_w^T]
is added by DVE. The two decoder hidden layers fuse z_dyn away entirely
via W = dec_w1 @ K_w. The P-step rollout uses K-power matrices
KPOW[:, 64(t-1):64t] = (K^T)^t built by doubling (lo half, then one
SBUF->SBUF DMA replicates to the hi half); x_pred staging is chunk-major
(one batch per partition -> 2KB-contiguous DRAM runs).
"""

import numpy as np

B, S, D, L, H, NCORES = 1024, 512, 32, 64, 64, 8
BS = B // NCORES          # 128 batches per core
NPAIR = BS // 2           # 64 pairs

_CACHE = {}


def _build(P, npair=NPAIR, do_main=True, do_pred=True):
    import concourse.mybir as mybir
    import concourse.tile as tile
    from concourse import bacc

    F32 = mybir.dt.float32
    F32R = mybir.dt.float32r
    AF = mybir.ActivationFunctionType
    ALU = mybir.AluOpType

    assert P % 16 == 0
    NTB = P // 16  # pred time blocks

    nc = bacc.Bacc("TRN2", target_bir_lowering=False, debug=False,
                   enable_asserts=True, num_devices=NCORES)

    xt_d = nc.dram_tensor("xt", [npair, 64, S], F32R, kind="ExternalInput").ap()
    ew1_d = nc.dram_tensor("enc_w1", [H, D], F32, kind="ExternalInput").ap()
    eb1_d = nc.dram_tensor("enc_b1", [H], F32, kind="ExternalInput").ap()
    ew2_d = nc.dram_tensor("enc_w2", [L, H], F32, kind="ExternalInput").ap()
    eb2_d = nc.dram_tensor("enc_b2", [L], F32, kind="ExternalInput").ap()
    dw1_d = nc.dram_tensor("dec_w1", [H, L], F32, kind="ExternalInput").ap()
    db1_d = nc.dram_tensor("dec_b1", [H], F32, kind="ExternalInput").ap()
    dw2_d = nc.dram_tensor("dec_w2", [D, H], F32, kind="ExternalInput").ap()
    db2_d = nc.dram_tensor("dec_b2", [D], F32, kind="ExternalInput").ap()
    kw_d = nc.dram_tensor("K_w", [L, L], F32, kind="ExternalInput").ap()

    xrec_d = nc.dram_tensor("x_rec", [2 * npair, S, D], F32, kind="ExternalOutput").ap()
    xdyn_d = nc.dram_tensor("x_dyn", [2 * npair, S, D], F32, kind="ExternalOutput").ap()
    xprd_d = nc.dram_tensor("x_pred", [2 * npair, P, D], F32, kind="ExternalOutput").ap()
    z_d = nc.dram_tensor("z", [2 * npair, S, L], F32, kind="ExternalOutput").ap()
    zdyn_d = nc.dram_tensor("z_dyn", [2 * npair, S, L], F32, kind="ExternalOutput").ap()

    LO, HI = slice(0, 64), slice(64, 128)

    def r32(ap):
        return ap.bitcast(F32R)

    with tile.TileContext(nc) as tc:
        with tc.tile_pool(name="const", bufs=1) as cp:
            # ---------------- weights / biases / constants ----------------
            # block-diagonal pair weights: [0:64,0:64]=W, [64:128,64:128]=W
            eblk = cp.tile([128, 128], F32R)   # enc_w1^T blocks at rows 0:32/32:64
            ew2blk = cp.tile([128, 128], F32R)  # enc_w2^T block-diag
            dwblk = cp.tile([128, 128], F32R)  # dec_w1^T block-diag
            wfblk = cp.tile([128, 128], F32R)  # (dec_w1 K_w)^T block-diag
            nc.gpsimd.memset(eblk[:, :].bitcast(F32), 0.0)
            nc.gpsimd.memset(ew2blk[:, :].bitcast(F32), 0.0)
            nc.gpsimd.memset(dwblk[:, :].bitcast(F32), 0.0)
            nc.gpsimd.memset(wfblk[:, :].bitcast(F32), 0.0)
            nc.sync.dma_start(eblk[0:32, 0:64], r32(ew1_d.rearrange("a b -> b a")))
            nc.sync.dma_start(eblk[32:64, 64:128], r32(ew1_d.rearrange("a b -> b a")))
            nc.sync.dma_start(ew2blk[LO, 0:64], r32(ew2_d.rearrange("a b -> b a")))
            nc.sync.dma_start(ew2blk[HI, 64:128], r32(ew2_d.rearrange("a b -> b a")))
            nc.sync.dma_start(dwblk[LO, 0:64], r32(dw1_d.rearrange("a b -> b a")))
            nc.sync.dma_start(dwblk[HI, 64:128], r32(dw1_d.rearrange("a b -> b a")))

            ew2s = cp.tile([128, H], F32R)    # enc_w2 straight (lo)
            dw1t = cp.tile([128, H], F32R)    # dec_w1^T lo/hi (pred + wf mm)
            dw2t = cp.tile([128, D], F32R)    # dec_w2^T lo/hi (flips)
            kwt = cp.tile([128, L], F32R)     # K_w^T lo
            kst = cp.tile([128, L], F32R)     # K_w straight lo
            nc.sync.dma_start(ew2s[0:64, :], r32(ew2_d[:, :]))
            nc.sync.dma_start(dw1t[LO, :], r32(dw1_d.rearrange("a b -> b a")))
            nc.sync.dma_start(dw1t[HI, :], r32(dw1_d.rearrange("a b -> b a")))
            nc.sync.dma_start(dw2t[LO, :], r32(dw2_d.rearrange("a b -> b a")))
            nc.sync.dma_start(dw2t[HI, :], r32(dw2_d.rearrange("a b -> b a")))
            nc.sync.dma_start(kwt[LO, :], r32(kw_d.rearrange("a b -> b a")))
            nc.sync.dma_start(kst[LO, :], r32(kw_d[:, :]))

            b1e = cp.tile([128, 1], F32)     # enc_b1 lo/hi (per-partition)
            b2e = cp.tile([128, 1], F32)     # enc_b2 lo/hi
            b1d = cp.tile([128, 1], F32)     # dec_b1 lo/hi
            for t_, src in ((b1e, eb1_d), (b2e, eb2_d), (b1d, db1_d)):
                nc.sync.dma_start(t_[LO, 0:1], src.rearrange("(a b) -> a b", b=1))
                nc.sync.dma_start(t_[HI, 0:1], src.rearrange("(a b) -> a b", b=1))
            b2er = cp.tile([128, 1], F32R)
            nc.vector.tensor_copy(b2er[LO, 0:1], b2e[LO, 0:1])

            # z/z_dyn flip rhs [enc_w2^T | EK | zero pad], lo+hi rows
            zzdw = cp.tile([128, 256], F32R)
            nc.gpsimd.memset(zzdw[:, :].bitcast(F32), 0.0)
            nc.sync.dma_start(zzdw[LO, 0:64], r32(ew2_d.rearrange("a b -> b a")))

            ones = cp.tile([128, 128], F32)
            nc.gpsimd.memset(ones[0:1, :], 1.0)
            zzd_row = cp.tile([128, 512], F32)  # 4x [enc_b2 | enc_b2 @ K_w^T]
            b2d_row = cp.tile([128, 512], F32)  # 16x dec_b2
            for r in range(4):
                nc.sync.dma_start(zzd_row[0:1, 128 * r:128 * r + 64],
                                  eb2_d.rearrange("(a b) -> a b", a=1))
            for r in range(16):
                nc.sync.dma_start(b2d_row[0:1, 32 * r:32 * r + 32],
                                  db2_d.rearrange("(a b) -> a b", a=1))
            zzd_rep = cp.tile([128, 512], F32)
            b2d_rep = cp.tile([128, 512], F32)
            bzrow = cp.tile([128, 64], F32)

            kpow = cp.tile([128, 64 * P], F32R)   # (K^T)^t at [:, 64(t-1):64t]
            zlast = cp.tile([128, L], F32R)       # col j = z[:,-1,:] of pair j
            qcur = cp.tile([128, L], F32R)        # K^m (doubling helper)
            qnxt = cp.tile([128, L], F32R)

            with tc.tile_pool(name="sps", bufs=2, space="PSUM") as sps:
                # EK (lo): EK[h,l'] = sum_l enc_w2[l,h] K^T[l,l']
                ekp = sps.tile([128, 512], F32, tag="s")
                nc.tensor.matmul(ekp[0:64, 0:64], ew2s[0:64, :], kwt[0:64, :],
                                 start=True, stop=True)
                nc.scalar.copy(zzdw[LO, 64:128], ekp[0:64, 0:64])
                # replicate zzdw lo rows -> hi rows (incl. zero pad)
                nc.sync.dma_start(zzdw[HI, :], zzdw[LO, :])

                # wf (lo): (dec_w1 K_w)^T = K_w^T dec_w1^T
                wfp = sps.tile([128, 512], F32, tag="s")
                nc.tensor.matmul(wfp[0:64, 0:64], kst[LO, :], dw1t[LO, :],
                                 start=True, stop=True)
                nc.scalar.copy(wfblk[LO, 0:64], wfp[0:64, 0:64])
                nc.sync.dma_start(wfblk[HI, 64:128], wfblk[LO, 0:64])

                # bzd row = enc_b2 @ K_w^T
                bzp = sps.tile([128, 512], F32, tag="s")
                nc.tensor.matmul(bzp[0:1, 0:64], b2er[LO, 0:1], kwt[LO, :],
                                 start=True, stop=True)
                nc.vector.tensor_copy(bzrow[0:1, :], bzp[0:1, 0:64])
                for r in range(4):
                    nc.vector.tensor_copy(zzd_row[0:1, 128 * r + 64:128 * r + 128],
                                          bzrow[0:1, :])

                # broadcast bias rows to all partitions via K=1 ones matmuls
                bp = sps.tile([128, 512], F32, tag="s")
                nc.tensor.matmul(bp[:, :], ones[0:1, 0:128], zzd_row[0:1, :],
                                 start=True, stop=True)
                nc.vector.tensor_copy(zzd_rep[:, :], bp[:, :])
                bp2 = sps.tile([128, 512], F32, tag="s")
                nc.tensor.matmul(bp2[:, :], ones[0:1, 0:128], b2d_row[0:1, :],
                                 start=True, stop=True)
                nc.vector.tensor_copy(b2d_rep[:, :], bp2[:, :])

                # ---------------- K powers (lo half only) ----------------
                nc.vector.tensor_copy(kpow[LO, 0:64], kwt[LO, :])  # P_1
                for t in range(1, 8):  # P_2..P_8 (serial)
                    pp = sps.tile([128, 512], F32, tag="s")
                    nc.tensor.matmul(pp[0:64, 0:64], kst[LO, :],
                                     kpow[LO, 64 * (t - 1):64 * t],
                                     start=True, stop=True)
                    nc.vector.tensor_copy(kpow[LO, 64 * t:64 * t + 64], pp[0:64, 0:64])
                nc.vector.tensor_copy(qcur[LO, :], kst[LO, :])  # Q_1
                for m in (1, 2, 4):   # Q_2, Q_4, Q_8
                    qp = sps.tile([128, 512], F32, tag="s")
                    nc.tensor.matmul(qp[0:64, 0:64],
                                     kpow[LO, 64 * (m - 1):64 * m],
                                     qcur[LO, :], start=True, stop=True)
                    nc.vector.tensor_copy(qnxt[LO, :], qp[0:64, 0:64])
                    qcur, qnxt = qnxt, qcur
                m = 8
                while m < P:  # S_{m+1..2m} = mm(lhsT=Q_m, rhs=S_{1..m})
                    for ch in range(max(1, (64 * m) // 512)):
                        n0 = 512 * ch if 64 * m > 512 else 0
                        nn = min(512, 64 * m)
                        dp = sps.tile([128, 512], F32, tag="s")
                        nc.tensor.matmul(dp[0:64, 0:nn], qcur[LO, :],
                                         kpow[LO, n0:n0 + nn],
                                         start=True, stop=True)
                        nc.vector.tensor_copy(kpow[LO, 64 * m + n0:64 * m + n0 + nn],
                                              dp[0:64, 0:nn])
                    if 2 * m < P:
                        qp = sps.tile([128, 512], F32, tag="s")
                        nc.tensor.matmul(qp[0:64, 0:64],
                                         kpow[LO, 64 * (m - 1):64 * m],
                                         qcur[LO, :], start=True, stop=True)
                        nc.vector.tensor_copy(qnxt[LO, :], qp[0:64, 0:64])
                        qcur, qnxt = qnxt, qcur
                    m *= 2
                # replicate powers to hi rows for the odd-batch rollout chain
                nc.sync.dma_start(kpow[HI, :], kpow[LO, :])

            # ---------------- main loop over batch pairs ----------------
            # PSUM budget (8 banks): mm x3, ztmA, ztmB, xpA, xpB (+1 spare)
            with tc.tile_pool(name="work", bufs=2) as wp_pool, \
                 tc.tile_pool(name="xin", bufs=3) as xin_pool, \
                 tc.tile_pool(name="stage", bufs=3) as st_pool, \
                 tc.tile_pool(name="mmp", bufs=2, space="PSUM") as mm_pool, \
                 tc.tile_pool(name="ztmp", bufs=2, space="PSUM") as ztm_pool, \
                 tc.tile_pool(name="tmp", bufs=1, space="PSUM") as tm_pool:

                for j in range(npair if do_main else 0):
                    bA = 2 * j
                    xT = xin_pool.tile([128, S], F32R, tag="xT")
                    nc.sync.dma_start(xT[0:64, :], xt_d[j])

                    # encoder layer 1: block-diag, rows 0:64 -> out pair-stacked
                    pre1 = mm_pool.tile([128, S], F32, tag="mm")
                    nc.tensor.matmul(pre1[:, :], eblk[0:64, :], xT[0:64, :],
                                     start=True, stop=True)
                    h1T = wp_pool.tile([128, S], F32R, tag="h1T")
                    nc.scalar.activation(h1T[:, :], pre1[:, :], AF.Relu, bias=b1e[:, 0:1])

                    # encoder layer 2 (K=128 block-diag)
                    zp = mm_pool.tile([128, S], F32, tag="mm")
                    nc.tensor.matmul(zp[:, :], ew2blk[:, :], h1T[:, :], start=True, stop=True)
                    zT = wp_pool.tile([128, S], F32R, tag="zT")
                    nc.scalar.activation(zT[:, :], zp[:, :], AF.Identity, bias=b2e[:, 0:1])

                    # decoder hidden on z and (fused) on z_dyn, pair-stacked
                    hz = mm_pool.tile([128, S], F32, tag="mm")
                    nc.tensor.matmul(hz[:, :], dwblk[:, :], zT[:, :], start=True, stop=True)
                    h1z = wp_pool.tile([128, S], F32R, tag="h1z")
                    nc.scalar.activation(h1z[:, :], hz[:, :], AF.Relu, bias=b1d[:, 0:1])
                    hd = mm_pool.tile([128, S], F32, tag="mm")
                    nc.tensor.matmul(hd[:, :], wfblk[:, :], zT[:, :], start=True, stop=True)
                    h1d = wp_pool.tile([128, S], F32R, tag="h1d")
                    nc.scalar.activation(h1d[:, :], hd[:, :], AF.Relu, bias=b1d[:, 0:1])

                    # ---- flip-mms (token-major, all dst partition 0) ----
                    # z/zdyn merged; chunks 0..2 zero-padded to N=256 (f32r full rate)
                    ztmA = ztm_pool.tile([128, 512], F32, tag="ztmA")
                    ztmB = ztm_pool.tile([128, 512], F32, tag="ztmB")
                    for c in range(4):
                        cs = slice(128 * c, 128 * c + 128)
                        nn = 256 if c < 3 else 128
                        nc.tensor.matmul(ztmA[:, 128 * c:128 * c + nn],
                                         h1T[LO, cs], zzdw[LO, 0:nn], start=True, stop=True)
                        nc.tensor.matmul(ztmB[:, 128 * c:128 * c + nn],
                                         h1T[HI, cs], zzdw[HI, 0:nn], start=True, stop=True)
                    # x_rec/x_dyn flips: bank per batch (A: rows 0:64, B: rows 64:128)
                    xpA = tm_pool.tile([128, 512], F32, tag="xpA")  # use [:, 0:256]
                    xpB = tm_pool.tile([128, 512], F32, tag="xpB")
                    for c in range(4):
                        cs = slice(128 * c, 128 * c + 128)
                        nc.tensor.matmul(xpA[:, 32 * c:32 * c + 32],
                                         h1z[LO, cs], dw2t[LO, :], start=True, stop=True)
                        nc.tensor.matmul(xpA[:, 128 + 32 * c:128 + 32 * c + 32],
                                         h1d[LO, cs], dw2t[LO, :], start=True, stop=True)
                        nc.tensor.matmul(xpB[:, 32 * c:32 * c + 32],
                                         h1z[HI, cs], dw2t[HI, :], start=True, stop=True)
                        nc.tensor.matmul(xpB[:, 128 + 32 * c:128 + 32 * c + 32],
                                         h1d[HI, cs], dw2t[HI, :], start=True, stop=True)

                    # bias adds into 2-pair staging tiles (stores batched
                    # over 4 batches to halve DMA count). Layouts put the
                    # output-tensor split OUTERMOST so DMA APs stay <=3 dims:
                    # zs2: [z (q c l) | zdyn (q c l)], xs4: [rec (q c d) | dyn ...]
                    GRP = 4  # pairs per store batch
                    if j % GRP == 0:
                        zs2 = st_pool.tile([128, 1024 * GRP], F32, tag="zs2")
                        xs4 = st_pool.tile([128, 512 * GRP], F32, tag="xs4")
                    zv2 = zs2[:, :].rearrange("p (w q c l) -> p w q c l", w=2, q=2 * GRP, c=4)
                    xv4 = xs4[:, :].rearrange("p (o q c d) -> p o q c d", o=2, q=2 * GRP, c=4)
                    qA, qB = 2 * (j % GRP), 2 * (j % GRP) + 1
                    nc.vector.tensor_tensor(
                        zv2[:, :, qA], ztmA[:, :].rearrange("p (c w l) -> p w c l", c=4, w=2),
                        zzd_rep[:, :].rearrange("p (c w l) -> p w c l", c=4, w=2), op=ALU.add)
                    nc.vector.tensor_tensor(
                        zv2[:, :, qB], ztmB[:, :].rearrange("p (c w l) -> p w c l", c=4, w=2),
                        zzd_rep[:, :].rearrange("p (c w l) -> p w c l", c=4, w=2), op=ALU.add)
                    nc.vector.tensor_tensor(
                        xv4[:, :, qA], xpA[:, 0:256].rearrange("p (o cd) -> p o cd", o=2),
                        b2d_rep[:, 0:256].rearrange("p (o cd) -> p o cd", o=2), op=ALU.add)
                    nc.vector.tensor_tensor(
                        xv4[:, :, qB], xpB[:, 0:256].rearrange("p (o cd) -> p o cd", o=2),
                        b2d_rep[:, 0:256].rearrange("p (o cd) -> p o cd", o=2), op=ALU.add)

                    # keep z[:, -1, :] column for the rollout
                    nc.vector.tensor_copy(zlast[:, j:j + 1], zT[:, S - 1:S])

                    # ---- stores (every second pair, 4 batches per DMA) ----
                    if j % GRP == GRP - 1 or j == npair - 1:
                        b0 = bA - 2 * (j % GRP)
                        nb = 2 * (j % GRP) + 2
                        nc.sync.dma_start(
                            z_d[b0:b0 + nb].rearrange("q (c p) l -> p q c l", p=128),
                            zs2[:, 0:256 * nb].rearrange("p (q c l) -> p q c l", q=nb, c=4))
                        nc.scalar.dma_start(
                            zdyn_d[b0:b0 + nb].rearrange("q (c p) l -> p q c l", p=128),
                            zs2[:, 512 * GRP:512 * GRP + 256 * nb].rearrange("p (q c l) -> p q c l", q=nb, c=4))
                        nc.sync.dma_start(
                            xrec_d[b0:b0 + nb].rearrange("q (c p) d -> p q c d", p=128),
                            xs4[:, 0:128 * nb].rearrange("p (q c d) -> p q c d", q=nb, c=4))
                        nc.scalar.dma_start(
                            xdyn_d[b0:b0 + nb].rearrange("q (c p) d -> p q c d", p=128),
                            xs4[:, 256 * GRP:256 * GRP + 128 * nb].rearrange("p (q c d) -> p q c d", q=nb, c=4))

            # ---------------- prediction rollout ----------------
            # chain A = even batches (zlast rows 0:64, kpow lo),
            # chain B = odd batches (rows 64:128, kpow hi); all dst partition 0.
            with tc.tile_pool(name="pwork", bufs=2) as pw_pool, \
                 tc.tile_pool(name="pmm", bufs=1, space="PSUM") as pmm_pool, \
                 tc.tile_pool(name="ph", bufs=1, space="PSUM") as ph_pool, \
                 tc.tile_pool(name="pacc", bufs=2, space="PSUM") as pacc_pool:
                xpr_v = xprd_d.rearrange("(b2 two) t d -> b2 two t d", two=2)
                npq = npair  # pair count = flip M
                for tb in range(NTB if do_pred else 0):
                    xppA = pacc_pool.tile([128, 512], F32, tag="xppA")
                    xppB = pacc_pool.tile([128, 512], F32, tag="xppB")
                    if npair < 64:
                        nc.vector.memset(xppA[:, :], 0.0)
                        nc.vector.memset(xppB[:, :], 0.0)
                    for sb in range(2):
                        t0 = 16 * tb + 8 * sb  # powers t0+1 .. t0+8
                        # zpred for 8 steps: 4 t-pair matmuls (M=128) per chain
                        zppA = pmm_pool.tile([128, 512], F32, tag="zppA")
                        zppB = pmm_pool.tile([128, 512], F32, tag="zppB")
                        if npair < 64:
                            nc.vector.memset(zppA[:, :], 0.0)
                            nc.vector.memset(zppB[:, :], 0.0)
                        for q in range(4):
                            ks = slice(64 * (t0 + 2 * q), 64 * (t0 + 2 * q) + 128)
                            nc.tensor.matmul(zppA[:, 64 * q:64 * q + npq],
                                             kpow[LO, ks], zlast[LO, 0:npq],
                                             start=True, stop=True)
                            nc.tensor.matmul(zppB[:, 64 * q:64 * q + npq],
                                             kpow[HI, ks], zlast[HI, 0:npq],
                                             start=True, stop=True)
                        # partitions of zpp: 0:64 = t-even l, 64:128 = t-odd l
                        zpsA = pw_pool.tile([128, 256], F32R, tag="zpsA")
                        zpsB = pw_pool.tile([128, 256], F32R, tag="zpsB")
                        nc.vector.tensor_copy(zpsA[:, :], zppA[:, 0:256])
                        nc.vector.tensor_copy(zpsB[:, :], zppB[:, 0:256])
                        # decoder hidden: parity e reads rows 0:64, parity o rows 64:128
                        hE = ph_pool.tile([128, 512], F32, tag="hE")
                        hO = ph_pool.tile([128, 512], F32, tag="hO")
                        nc.tensor.matmul(hE[0:64, 0:256], dw1t[LO, :], zpsA[LO, :],
                                         start=True, stop=True)
                        nc.tensor.matmul(hO[0:64, 0:256], dw1t[HI, :], zpsA[HI, :],
                                         start=True, stop=True)
                        nc.tensor.matmul(hE[0:64, 256:512], dw1t[LO, :], zpsB[LO, :],
                                         start=True, stop=True)
                        nc.tensor.matmul(hO[0:64, 256:512], dw1t[HI, :], zpsB[HI, :],
                                         start=True, stop=True)
                        h1E = pw_pool.tile([128, 512], F32R, tag="h1E")
                        h1O = pw_pool.tile([128, 512], F32R, tag="h1O")
                        nc.scalar.activation(h1E[0:64, :], hE[0:64, :], AF.Relu, bias=b1d[LO, 0:1])
                        nc.scalar.activation(h1O[0:64, :], hO[0:64, :], AF.Relu, bias=b1d[LO, 0:1])
                        # flips: t = t0 + 2q + par; chain A cols 0:256, B cols 256:512
                        for q in range(4):
                            for par, h1 in ((0, h1E), (1, h1O)):
                                tloc = 8 * sb + 2 * q + par
                                nc.tensor.matmul(
                                    xppA[0:npq, 32 * tloc:32 * tloc + 32],
                                    h1[0:64, 64 * q:64 * q + npq], dw2t[LO, :],
                                    start=True, stop=True)
                                nc.tensor.matmul(
                                    xppB[0:npq, 32 * tloc:32 * tloc + 32],
                                    h1[0:64, 256 + 64 * q:256 + 64 * q + npq], dw2t[LO, :],
                                    start=True, stop=True)
                    xpsA = pw_pool.tile([128, 512], F32, tag="xpsA")
                    xpsB = pw_pool.tile([128, 512], F32, tag="xpsB")
                    nc.vector.tensor_tensor(xpsA[0:64, :], xppA[0:64, :], b2d_rep[0:64, :], op=ALU.add)
                    nc.vector.tensor_tensor(xpsB[0:64, :], xppB[0:64, :], b2d_rep[0:64, :], op=ALU.add)
                    nc.sync.dma_start(
                        xpr_v[:, 0, 16 * tb:16 * tb + 16, :],
                        xpsA[0:npq, :].rearrange("p (t d) -> p t d", t=16))
                    nc.scalar.dma_start(
                        xpr_v[:, 1, 16 * tb:16 * tb + 16, :],
                        xpsB[0:npq, :].rearrange("p (t d) -> p t d", t=16))

    nc.compile()
    return nc


def _prep_xt(x_shard):
    # [128, 512, 32] -> [64 pairs, 64, 512]; rows 0:32 even batch x^T, 32:64 odd
    xt = np.empty((NPAIR, 64, S), np.float32)
    xt[:, 0:32, :] = x_shard[0::2].transpose(0, 2, 1)
    xt[:, 32:64, :] = x_shard[1::2].transpose(0, 2, 1)
    return xt


def _run(inputs, trace=False):
    from concourse import bass_utils

    x = np.asarray(inputs["x"], np.float32)
    P = int(inputs["pred_len"])
    key = P
    if key not in _CACHE:
        _CACHE[key] = _build(P)
    nc = _CACHE[key]

    weights = {k: np.ascontiguousarray(np.asarray(inputs[k], np.float32))
               for k in ("enc_w1", "enc_b1", "enc_w2", "enc_b2",
                         "dec_w1", "dec_b1", "dec_w2", "dec_b2", "K_w")}
    in_maps = []
    for c in range(NCORES):
        m = dict(weights)
        m["xt"] = _prep_xt(x[c * BS:(c + 1) * BS])
        in_maps.append(m)

    res = bass_utils.run_bass_kernel_spmd(nc, in_maps, core_ids=list(range(NCORES)),
                                          trace=trace)
    rs = res.results
    x_rec = np.concatenate([r["x_rec"] for r in rs], 0)
    x_dyn = np.concatenate([r["x_dyn"] for r in rs], 0)
    x_pred = np.concatenate([r["x_pred"] for r in rs], 0)
    z = np.concatenate([r["z"] for r in rs], 0)
    z_dyn = np.concatenate([r["z_dyn"] for r in rs], 0)
    return (x_rec, x_dyn, x_pred, z, z_dyn), res


def kernel(**inputs):
    return _run(inputs)[0]


# revision 33
# speedup vs baseline: 2.0128x; 1.0217x over previous
"""DeepKoopman Trainium2 kernel (8-core data-parallel).

Per core (128-batch shard): activations ride the free dim in "T-form"
(features on partitions); each 512-token chunk is one batch row (S=512).
Batch pairs (A, B) are stacked on partition halves of [128, 512] tiles.

Matmuls use float32r (TF32-like, ~2e-4 rel err, 4x the fp32 PE rate at
N>=256). HW rules discovered on the way (violations lock up the device or
fail codegen):
  - f32r matmul outputs MUST start at PSUM partition 0 (input row strips
    are free) -> batch-pair matmuls use BLOCK-DIAGONAL weights (K=128,
    M=128, dst 0:128) instead of two half-matmuls.
  - Matmuls that can run concurrently (disjoint PE row strips) must write
    DIFFERENT PSUM BANKS - concurrent same-bank drains are fatal.
  - PSUM pool slots are not bank-aligned; tiles are explicitly bank-sized.
Token-major outputs come from "flip" matmuls (lhsT = activationT slice).
z and z_dyn share one flip via rhs = [enc_w2^T | enc_w2^T K_w^T] with a
zero pad to N=256 so f32r runs at full rate; bias [enc_b2 | enc_b2 K_w^T]
is added by DVE. The two decoder hidden layers fuse z_dyn away entirely
via W = dec_w1 @ K_w. The P-step rollout uses K-power matrices
KPOW[:, 64(t-1):64t] = (K^T)^t built by doubling (lo half, then one
SBUF->SBUF DMA replicates to the hi half); x_pred staging is chunk-major
(one batch per partition -> 2KB-contiguous DRAM runs).
"""

import numpy as np

B, S, D, L, H, NCORES = 1024, 512, 32, 64, 64, 8
BS = B // NCORES          # 128 batches per core
NPAIR = BS // 2           # 64 pairs

_CACHE = {}


def _build(P, npair=NPAIR, do_main=True, do_pred=True):
    import concourse.mybir as mybir
    import concourse.tile as tile
    from concourse import bacc

    F32 = mybir.dt.float32
    F32R = mybir.dt.float32r
    AF = mybir.ActivationFunctionType
    ALU = mybir.AluOpType

    assert P % 16 == 0
    NTB = P // 16  # pred time blocks

    nc = bacc.Bacc("TRN2", target_bir_lowering=False, debug=False,
                   enable_asserts=True, num_devices=NCORES)

    xt_d = nc.dram_tensor("xt", [npair, 64, S], F32R, kind="ExternalInput").ap()
    ew1_d = nc.dram_tensor("enc_w1", [H, D], F32, kind="ExternalInput").ap()
    eb1_d = nc.dram_tensor("enc_b1", [H], F32, kind="ExternalInput").ap()
    ew2_d = nc.dram_tensor("enc_w2", [L, H], F32, kind="ExternalInput").ap()
    eb2_d = nc.dram_tensor("enc_b2", [L], F32, kind="ExternalInput").ap()
    dw1_d = nc.dram_tensor("dec_w1", [H, L], F32, kind="ExternalInput").ap()
    db1_d = nc.dram_tensor("dec_b1", [H], F32, kind="ExternalInput").ap()
    dw2_d = nc.dram_tensor("dec_w2", [D, H], F32, kind="ExternalInput").ap()
    db2_d = nc.dram_tensor("dec_b2", [D], F32, kind="ExternalInput").ap()
    kw_d = nc.dram_tensor("K_w", [L, L], F32, kind="ExternalInput").ap()

    xrec_d = nc.dram_tensor("x_rec", [2 * npair, S, D], F32, kind="ExternalOutput").ap()
    xdyn_d = nc.dram_tensor("x_dyn", [2 * npair, S, D], F32, kind="ExternalOutput").ap()
    xprd_d = nc.dram_tensor("x_pred", [2 * npair, P, D], F32, kind="ExternalOutput").ap()
    z_d = nc.dram_tensor("z", [2 * npair, S, L], F32, kind="ExternalOutput").ap()
    zdyn_d = nc.dram_tensor("z_dyn", [2 * npair, S, L], F32, kind="ExternalOutput").ap()

    LO, HI = slice(0, 64), slice(64, 128)

    def r32(ap):
        return ap.bitcast(F32R)

    with tile.TileContext(nc) as tc:
        with tc.tile_pool(name="const", bufs=1) as cp:
            # ---------------- weights / biases / constants ----------------
            # block-diagonal pair weights: [0:64,0:64]=W, [64:128,64:128]=W
            eblk = cp.tile([128, 128], F32R)   # enc_w1^T blocks at rows 0:32/32:64
            ew2blk = cp.tile([128, 128], F32R)  # enc_w2^T block-diag
            dwblk = cp.tile([128, 128], F32R)  # dec_w1^T block-diag
            wfblk = cp.tile([128, 128], F32R)  # (dec_w1 K_w)^T block-diag
            nc.gpsimd.memset(eblk[:, :].bitcast(F32), 0.0)
            nc.gpsimd.memset(ew2blk[:, :].bitcast(F32), 0.0)
            nc.gpsimd.memset(dwblk[:, :].bitcast(F32), 0.0)
            nc.gpsimd.memset(wfblk[:, :].bitcast(F32), 0.0)
            nc.sync.dma_start(eblk[0:32, 0:64], r32(ew1_d.rearrange("a b -> b a")))
            nc.sync.dma_start(eblk[32:64, 64:128], r32(ew1_d.rearrange("a b -> b a")))
            nc.sync.dma_start(ew2blk[LO, 0:64], r32(ew2_d.rearrange("a b -> b a")))
            nc.sync.dma_start(ew2blk[HI, 64:128], r32(ew2_d.rearrange("a b -> b a")))
            nc.sync.dma_start(dwblk[LO, 0:64], r32(dw1_d.rearrange("a b -> b a")))
            nc.sync.dma_start(dwblk[HI, 64:128], r32(dw1_d.rearrange("a b -> b a")))

            ew2s = cp.tile([128, H], F32R)    # enc_w2 straight (lo)
            dw1t = cp.tile([128, H], F32R)    # dec_w1^T lo/hi (pred + wf mm)
            dw2t = cp.tile([128, D], F32R)    # dec_w2^T lo/hi (flips)
            kwt = cp.tile([128, L], F32R)     # K_w^T lo
            kst = cp.tile([128, L], F32R)     # K_w straight lo
            nc.sync.dma_start(ew2s[0:64, :], r32(ew2_d[:, :]))
            nc.sync.dma_start(dw1t[LO, :], r32(dw1_d.rearrange("a b -> b a")))
            nc.sync.dma_start(dw1t[HI, :], r32(dw1_d.rearrange("a b -> b a")))
            nc.sync.dma_start(dw2t[LO, :], r32(dw2_d.rearrange("a b -> b a")))
            nc.sync.dma_start(dw2t[HI, :], r32(dw2_d.rearrange("a b -> b a")))
            nc.sync.dma_start(kwt[LO, :], r32(kw_d.rearrange("a b -> b a")))
            nc.sync.dma_start(kst[LO, :], r32(kw_d[:, :]))

            b1e = cp.tile([128, 1], F32)     # enc_b1 lo/hi (per-partition)
            b2e = cp.tile([128, 1], F32)     # enc_b2 lo/hi
            b1d = cp.tile([128, 1], F32)     # dec_b1 lo/hi
            for t_, src in ((b1e, eb1_d), (b2e, eb2_d), (b1d, db1_d)):
                nc.sync.dma_start(t_[LO, 0:1], src.rearrange("(a b) -> a b", b=1))
                nc.sync.dma_start(t_[HI, 0:1], src.rearrange("(a b) -> a b", b=1))
            b2er = cp.tile([128, 1], F32R)
            nc.vector.tensor_copy(b2er[LO, 0:1], b2e[LO, 0:1])

            # z/z_dyn flip rhs [enc_w2^T | EK | zero pad], lo+hi rows
            zzdw = cp.tile([128, 256], F32R)
            nc.gpsimd.memset(zzdw[:, :].bitcast(F32), 0.0)
            nc.sync.dma_start(zzdw[LO, 0:64], r32(ew2_d.rearrange("a b -> b a")))

            ones = cp.tile([128, 128], F32)
            nc.gpsimd.memset(ones[0:1, :], 1.0)
            zzd_row = cp.tile([128, 512], F32)  # 4x [enc_b2 | enc_b2 @ K_w^T]
            b2d_row = cp.tile([128, 512], F32)  # 16x dec_b2
            for r in range(4):
                nc.sync.dma_start(zzd_row[0:1, 128 * r:128 * r + 64],
                                  eb2_d.rearrange("(a b) -> a b", a=1))
            for r in range(16):
                nc.sync.dma_start(b2d_row[0:1, 32 * r:32 * r + 32],
                                  db2_d.rearrange("(a b) -> a b", a=1))
            zzd_rep = cp.tile([128, 512], F32)
            b2d_rep = cp.tile([128, 512], F32)
            bzrow = cp.tile([128, 64], F32)

            kpow = cp.tile([128, 64 * P], F32R)   # (K^T)^t at [:, 64(t-1):64t]
            zlast = cp.tile([128, L], F32R)       # col j = z[:,-1,:] of pair j
            qcur = cp.tile([128, L], F32R)        # K^m (doubling helper)
            qnxt = cp.tile([128, L], F32R)

            with tc.tile_pool(name="sps", bufs=2, space="PSUM") as sps:
                # EK (lo): EK[h,l'] = sum_l enc_w2[l,h] K^T[l,l']
                ekp = sps.tile([128, 512], F32, tag="s")
                nc.tensor.matmul(ekp[0:64, 0:64], ew2s[0:64, :], kwt[0:64, :],
                                 start=True, stop=True)
                nc.scalar.copy(zzdw[LO, 64:128], ekp[0:64, 0:64])
                # replicate zzdw lo rows -> hi rows (incl. zero pad)
                nc.sync.dma_start(zzdw[HI, :], zzdw[LO, :])

                # wf (lo): (dec_w1 K_w)^T = K_w^T dec_w1^T
                wfp = sps.tile([128, 512], F32, tag="s")
                nc.tensor.matmul(wfp[0:64, 0:64], kst[LO, :], dw1t[LO, :],
                                 start=True, stop=True)
                nc.scalar.copy(wfblk[LO, 0:64], wfp[0:64, 0:64])
                nc.sync.dma_start(wfblk[HI, 64:128], wfblk[LO, 0:64])

                # bzd row = enc_b2 @ K_w^T
                bzp = sps.tile([128, 512], F32, tag="s")
                nc.tensor.matmul(bzp[0:1, 0:64], b2er[LO, 0:1], kwt[LO, :],
                                 start=True, stop=True)
                nc.vector.tensor_copy(bzrow[0:1, :], bzp[0:1, 0:64])
                for r in range(4):
                    nc.vector.tensor_copy(zzd_row[0:1, 128 * r + 64:128 * r + 128],
                                          bzrow[0:1, :])

                # broadcast bias rows to all partitions via K=1 ones matmuls
                bp = sps.tile([128, 512], F32, tag="s")
                nc.tensor.matmul(bp[:, :], ones[0:1, 0:128], zzd_row[0:1, :],
                                 start=True, stop=True)
                nc.vector.tensor_copy(zzd_rep[:, :], bp[:, :])
                bp2 = sps.tile([128, 512], F32, tag="s")
                nc.tensor.matmul(bp2[:, :], ones[0:1, 0:128], b2d_row[0:1, :],
                                 start=True, stop=True)
                nc.vector.tensor_copy(b2d_rep[:, :], bp2[:, :])

                # ---------------- K powers (lo half only) ----------------
                nc.vector.tensor_copy(kpow[LO, 0:64], kwt[LO, :])  # P_1
                for t in range(1, 8):  # P_2..P_8 (serial)
                    pp = sps.tile([128, 512], F32, tag="s")
                    nc.tensor.matmul(pp[0:64, 0:64], kst[LO, :],
                                     kpow[LO, 64 * (t - 1):64 * t],
                                     start=True, stop=True)
                    nc.vector.tensor_copy(kpow[LO, 64 * t:64 * t + 64], pp[0:64, 0:64])
                nc.vector.tensor_copy(qcur[LO, :], kst[LO, :])  # Q_1
                for m in (1, 2, 4):   # Q_2, Q_4, Q_8
                    qp = sps.tile([128, 512], F32, tag="s")
                    nc.tensor.matmul(qp[0:64, 0:64],
                                     kpow[LO, 64 * (m - 1):64 * m],
                                     qcur[LO, :], start=True, stop=True)
                    nc.vector.tensor_copy(qnxt[LO, :], qp[0:64, 0:64])
                    qcur, qnxt = qnxt, qcur
                m = 8
                while m < P:  # S_{m+1..2m} = mm(lhsT=Q_m, rhs=S_{1..m})
                    for ch in range(max(1, (64 * m) // 512)):
                        n0 = 512 * ch if 64 * m > 512 else 0
                        nn = min(512, 64 * m)
                        dp = sps.tile([128, 512], F32, tag="s")
                        nc.tensor.matmul(dp[0:64, 0:nn], qcur[LO, :],
                                         kpow[LO, n0:n0 + nn],
                                         start=True, stop=True)
                        nc.vector.tensor_copy(kpow[LO, 64 * m + n0:64 * m + n0 + nn],
                                              dp[0:64, 0:nn])
                    if 2 * m < P:
                        qp = sps.tile([128, 512], F32, tag="s")
                        nc.tensor.matmul(qp[0:64, 0:64],
                                         kpow[LO, 64 * (m - 1):64 * m],
                                         qcur[LO, :], start=True, stop=True)
                        nc.vector.tensor_copy(qnxt[LO, :], qp[0:64, 0:64])
                        qcur, qnxt = qnxt, qcur
                    m *= 2
                # replicate powers to hi rows for the odd-batch rollout chain
                nc.sync.dma_start(kpow[HI, :], kpow[LO, :])

            # ---------------- main loop over batch pairs ----------------
            # PSUM budget (8 banks): mm x3, ztmA, ztmB, xpA, xpB (+1 spare)
            with tc.tile_pool(name="work", bufs=3) as wp_pool, \
                 tc.tile_pool(name="xin", bufs=6) as xin_pool, \
                 tc.tile_pool(name="stage", bufs=3) as st_pool, \
                 tc.tile_pool(name="mmp", bufs=2, space="PSUM") as mm_pool, \
                 tc.tile_pool(name="ztmp", bufs=2, space="PSUM") as ztm_pool, \
                 tc.tile_pool(name="tmp", bufs=1, space="PSUM") as tm_pool:

                for j in range(npair if do_main else 0):
                    bA = 2 * j
                    xT = xin_pool.tile([128, S], F32R, tag="xT")
                    nc.gpsimd.dma_start(xT[0:64, :], xt_d[j])

                    # encoder layer 1: block-diag, rows 0:64 -> out pair-stacked
                    pre1 = mm_pool.tile([128, S], F32, tag="mm")
                    nc.tensor.matmul(pre1[:, :], eblk[0:64, :], xT[0:64, :],
                                     start=True, stop=True)
                    h1T = wp_pool.tile([128, S], F32R, tag="h1T")
                    nc.scalar.activation(h1T[:, :], pre1[:, :], AF.Relu, bias=b1e[:, 0:1])

                    # encoder layer 2 (K=128 block-diag)
                    zp = mm_pool.tile([128, S], F32, tag="mm")
                    nc.tensor.matmul(zp[:, :], ew2blk[:, :], h1T[:, :], start=True, stop=True)
                    zT = wp_pool.tile([128, S], F32R, tag="zT")
                    nc.scalar.activation(zT[:, :], zp[:, :], AF.Identity, bias=b2e[:, 0:1])

                    # decoder hidden on z and (fused) on z_dyn, pair-stacked
                    hz = mm_pool.tile([128, S], F32, tag="mm")
                    nc.tensor.matmul(hz[:, :], dwblk[:, :], zT[:, :], start=True, stop=True)
                    h1z = wp_pool.tile([128, S], F32R, tag="h1z")
                    nc.scalar.activation(h1z[:, :], hz[:, :], AF.Relu, bias=b1d[:, 0:1])
                    hd = mm_pool.tile([128, S], F32, tag="mm")
                    nc.tensor.matmul(hd[:, :], wfblk[:, :], zT[:, :], start=True, stop=True)
                    h1d = wp_pool.tile([128, S], F32R, tag="h1d")
                    nc.scalar.activation(h1d[:, :], hd[:, :], AF.Relu, bias=b1d[:, 0:1])

                    # ---- flip-mms (token-major, all dst partition 0) ----
                    # z/zdyn merged; chunks 0..2 zero-padded to N=256 (f32r full rate)
                    ztmA = ztm_pool.tile([128, 512], F32, tag="ztmA")
                    ztmB = ztm_pool.tile([128, 512], F32, tag="ztmB")
                    for c in range(4):
                        cs = slice(128 * c, 128 * c + 128)
                        nn = 256 if c < 3 else 128
                        nc.tensor.matmul(ztmA[:, 128 * c:128 * c + nn],
                                         h1T[LO, cs], zzdw[LO, 0:nn], start=True, stop=True)
                        nc.tensor.matmul(ztmB[:, 128 * c:128 * c + nn],
                                         h1T[HI, cs], zzdw[HI, 0:nn], start=True, stop=True)
                    # x_rec/x_dyn flips: bank per batch (A: rows 0:64, B: rows 64:128)
                    xpA = tm_pool.tile([128, 512], F32, tag="xpA")  # use [:, 0:256]
                    xpB = tm_pool.tile([128, 512], F32, tag="xpB")
                    for c in range(4):
                        cs = slice(128 * c, 128 * c + 128)
                        nc.tensor.matmul(xpA[:, 32 * c:32 * c + 32],
                                         h1z[LO, cs], dw2t[LO, :], start=True, stop=True)
                        nc.tensor.matmul(xpA[:, 128 + 32 * c:128 + 32 * c + 32],
                                         h1d[LO, cs], dw2t[LO, :], start=True, stop=True)
                        nc.tensor.matmul(xpB[:, 32 * c:32 * c + 32],
                                         h1z[HI, cs], dw2t[HI, :], start=True, stop=True)
                        nc.tensor.matmul(xpB[:, 128 + 32 * c:128 + 32 * c + 32],
                                         h1d[HI, cs], dw2t[HI, :], start=True, stop=True)

                    # bias adds into 2-pair staging tiles (stores batched
                    # over 4 batches to halve DMA count). Layouts put the
                    # output-tensor split OUTERMOST so DMA APs stay <=3 dims:
                    # zs2: [z (q c l) | zdyn (q c l)], xs4: [rec (q c d) | dyn ...]
                    GRP = 4  # pairs per store batch
                    if j % GRP == 0:
                        zs2 = st_pool.tile([128, 1024 * GRP], F32, tag="zs2")
                        xs4 = st_pool.tile([128, 512 * GRP], F32, tag="xs4")
                    zv2 = zs2[:, :].rearrange("p (w q c l) -> p w q c l", w=2, q=2 * GRP, c=4)
                    xv4 = xs4[:, :].rearrange("p (o q c d) -> p o q c d", o=2, q=2 * GRP, c=4)
                    qA, qB = 2 * (j % GRP), 2 * (j % GRP) + 1
                    nc.vector.tensor_tensor(
                        zv2[:, :, qA], ztmA[:, :].rearrange("p (c w l) -> p w c l", c=4, w=2),
                        zzd_rep[:, :].rearrange("p (c w l) -> p w c l", c=4, w=2), op=ALU.add)
                    nc.vector.tensor_tensor(
                        zv2[:, :, qB], ztmB[:, :].rearrange("p (c w l) -> p w c l", c=4, w=2),
                        zzd_rep[:, :].rearrange("p (c w l) -> p w c l", c=4, w=2), op=ALU.add)
                    nc.vector.tensor_tensor(
                        xv4[:, :, qA], xpA[:, 0:256].rearrange("p (o cd) -> p o cd", o=2),
                        b2d_rep[:, 0:256].rearrange("p (o cd) -> p o cd", o=2), op=ALU.add)
                    nc.vector.tensor_tensor(
                        xv4[:, :, qB], xpB[:, 0:256].rearrange("p (o cd) -> p o cd", o=2),
                        b2d_rep[:, 0:256].rearrange("p (o cd) -> p o cd", o=2), op=ALU.add)

                    # keep z[:, -1, :] column for the rollout
                    nc.vector.tensor_copy(zlast[:, j:j + 1], zT[:, S - 1:S])

                    # ---- stores (every second pair, 4 batches per DMA) ----
                    if j % GRP == GRP - 1 or j == npair - 1:
                        b0 = bA - 2 * (j % GRP)
                        nb = 2 * (j % GRP) + 2
                        nc.sync.dma_start(
                            z_d[b0:b0 + nb].rearrange("q (c p) l -> p q c l", p=128),
                            zs2[:, 0:256 * nb].rearrange("p (q c l) -> p q c l", q=nb, c=4))
                        nc.scalar.dma_start(
                            zdyn_d[b0:b0 + nb].rearrange("q (c p) l -> p q c l", p=128),
                            zs2[:, 512 * GRP:512 * GRP + 256 * nb].rearrange("p (q c l) -> p q c l", q=nb, c=4))
                        nc.sync.dma_start(
                            xrec_d[b0:b0 + nb].rearrange("q (c p) d -> p q c d", p=128),
                            xs4[:, 0:128 * nb].rearrange("p (q c d) -> p q c d", q=nb, c=4))
                        nc.scalar.dma_start(
                            xdyn_d[b0:b0 + nb].rearrange("q (c p) d -> p q c d", p=128),
                            xs4[:, 256 * GRP:256 * GRP + 128 * nb].rearrange("p (q c d) -> p q c d", q=nb, c=4))

            # ---------------- prediction rollout ----------------
            # chain A = even batches (zlast rows 0:64, kpow lo),
            # chain B = odd batches (rows 64:128, kpow hi); all dst partition 0.
            with tc.tile_pool(name="pwork", bufs=2) as pw_pool, \
                 tc.tile_pool(name="pmm", bufs=1, space="PSUM") as pmm_pool, \
                 tc.tile_pool(name="ph", bufs=1, space="PSUM") as ph_pool, \
                 tc.tile_pool(name="pacc", bufs=2, space="PSUM") as pacc_pool:
                xpr_v = xprd_d.rearrange("(b2 two) t d -> b2 two t d", two=2)
                npq = npair  # pair count = flip M
                for tb in range(NTB if do_pred else 0):
                    xppA = pacc_pool.tile([128, 512], F32, tag="xppA")
                    xppB = pacc_pool.tile([128, 512], F32, tag="xppB")
                    if npair < 64:
                        nc.vector.memset(xppA[:, :], 0.0)
                        nc.vector.memset(xppB[:, :], 0.0)
                    for sb in range(2):
                        t0 = 16 * tb + 8 * sb  # powers t0+1 .. t0+8
                        # zpred for 8 steps: 4 t-pair matmuls (M=128) per chain
                        zppA = pmm_pool.tile([128, 512], F32, tag="zppA")
                        zppB = pmm_pool.tile([128, 512], F32, tag="zppB")
                        if npair < 64:
                            nc.vector.memset(zppA[:, :], 0.0)
                            nc.vector.memset(zppB[:, :], 0.0)
                        for q in range(4):
                            ks = slice(64 * (t0 + 2 * q), 64 * (t0 + 2 * q) + 128)
                            nc.tensor.matmul(zppA[:, 64 * q:64 * q + npq],
                                             kpow[LO, ks], zlast[LO, 0:npq],
                                             start=True, stop=True)
                            nc.tensor.matmul(zppB[:, 64 * q:64 * q + npq],
                                             kpow[HI, ks], zlast[HI, 0:npq],
                                             start=True, stop=True)
                        # partitions of zpp: 0:64 = t-even l, 64:128 = t-odd l
                        zpsA = pw_pool.tile([128, 256], F32R, tag="zpsA")
                        zpsB = pw_pool.tile([128, 256], F32R, tag="zpsB")
                        nc.vector.tensor_copy(zpsA[:, :], zppA[:, 0:256])
                        nc.vector.tensor_copy(zpsB[:, :], zppB[:, 0:256])
                        # decoder hidden: parity e reads rows 0:64, parity o rows 64:128
                        hE = ph_pool.tile([128, 512], F32, tag="hE")
                        hO = ph_pool.tile([128, 512], F32, tag="hO")
                        nc.tensor.matmul(hE[0:64, 0:256], dw1t[LO, :], zpsA[LO, :],
                                         start=True, stop=True)
                        nc.tensor.matmul(hO[0:64, 0:256], dw1t[HI, :], zpsA[HI, :],
                                         start=True, stop=True)
                        nc.tensor.matmul(hE[0:64, 256:512], dw1t[LO, :], zpsB[LO, :],
                                         start=True, stop=True)
                        nc.tensor.matmul(hO[0:64, 256:512], dw1t[HI, :], zpsB[HI, :],
                                         start=True, stop=True)
                        h1E = pw_pool.tile([128, 512], F32R, tag="h1E")
                        h1O = pw_pool.tile([128, 512], F32R, tag="h1O")
                        nc.scalar.activation(h1E[0:64, :], hE[0:64, :], AF.Relu, bias=b1d[LO, 0:1])
                        nc.scalar.activation(h1O[0:64, :], hO[0:64, :], AF.Relu, bias=b1d[LO, 0:1])
                        # flips: t = t0 + 2q + par; chain A cols 0:256, B cols 256:512
                        for q in range(4):
                            for par, h1 in ((0, h1E), (1, h1O)):
                                tloc = 8 * sb + 2 * q + par
                                nc.tensor.matmul(
                                    xppA[0:npq, 32 * tloc:32 * tloc + 32],
                                    h1[0:64, 64 * q:64 * q + npq], dw2t[LO, :],
                                    start=True, stop=True)
                                nc.tensor.matmul(
                                    xppB[0:npq, 32 * tloc:32 * tloc + 32],
                                    h1[0:64, 256 + 64 * q:256 + 64 * q + npq], dw2t[LO, :],
                                    start=True, stop=True)
                    xpsA = pw_pool.tile([128, 512], F32, tag="xpsA")
                    xpsB = pw_pool.tile([128, 512], F32, tag="xpsB")
                    nc.vector.tensor_tensor(xpsA[0:64, :], xppA[0:64, :], b2d_rep[0:64, :], op=ALU.add)
                    nc.vector.tensor_tensor(xpsB[0:64, :], xppB[0:64, :], b2d_rep[0:64, :], op=ALU.add)
                    nc.sync.dma_start(
                        xpr_v[:, 0, 16 * tb:16 * tb + 16, :],
                        xpsA[0:npq, :].rearrange("p (t d) -> p t d", t=16))
                    nc.scalar.dma_start(
                        xpr_v[:, 1, 16 * tb:16 * tb + 16, :],
                        xpsB[0:npq, :].rearrange("p (t d) -> p t d", t=16))

    nc.compile()
    return nc


def _prep_xt(x_shard):
    # [128, 512, 32] -> [64 pairs, 64, 512]; rows 0:32 even batch x^T, 32:64 odd
    xt = np.empty((NPAIR, 64, S), np.float32)
    xt[:, 0:32, :] = x_shard[0::2].transpose(0, 2, 1)
    xt[:, 32:64, :] = x_shard[1::2].transpose(0, 2, 1)
    return xt


def _run(inputs, trace=False):
    from concourse import bass_utils

    x = np.asarray(inputs["x"], np.float32)
    P = int(inputs["pred_len"])
    key = P
    if key not in _CACHE:
        _CACHE[key] = _build(P)
    nc = _CACHE[key]

    weights = {k: np.ascontiguousarray(np.asarray(inputs[k], np.float32))
               for k in ("enc_w1", "enc_b1", "enc_w2", "enc_b2",
                         "dec_w1", "dec_b1", "dec_w2", "dec_b2", "K_w")}
    in_maps = []
    for c in range(NCORES):
        m = dict(weights)
        m["xt"] = _prep_xt(x[c * BS:(c + 1) * BS])
        in_maps.append(m)

    res = bass_utils.run_bass_kernel_spmd(nc, in_maps, core_ids=list(range(NCORES)),
                                          trace=trace)
    rs = res.results
    x_rec = np.concatenate([r["x_rec"] for r in rs], 0)
    x_dyn = np.concatenate([r["x_dyn"] for r in rs], 0)
    x_pred = np.concatenate([r["x_pred"] for r in rs], 0)
    z = np.concatenate([r["z"] for r in rs], 0)
    z_dyn = np.concatenate([r["z_dyn"] for r in rs], 0)
    return (x_rec, x_dyn, x_pred, z, z_dyn), res


def kernel(**inputs):
    return _run(inputs)[0]


# revision 39
# speedup vs baseline: 2.0161x; 1.0016x over previous
"""DeepKoopman Trainium2 kernel (8-core data-parallel).

Per core (128-batch shard): activations ride the free dim in "T-form"
(features on partitions); each 512-token chunk is one batch row (S=512).
Batch pairs (A, B) are stacked on partition halves of [128, 512] tiles.

Matmuls use float32r (TF32-like, ~2e-4 rel err, 4x the fp32 PE rate at
N>=256). HW rules discovered on the way (violations lock up the device or
fail codegen):
  - f32r matmul outputs MUST start at PSUM partition 0 (input row strips
    are free) -> batch-pair matmuls use BLOCK-DIAGONAL weights (K=128,
    M=128, dst 0:128) instead of two half-matmuls.
  - Matmuls that can run concurrently (disjoint PE row strips) must write
    DIFFERENT PSUM BANKS - concurrent same-bank drains are fatal.
  - PSUM pool slots are not bank-aligned; tiles are explicitly bank-sized.
Token-major outputs come from "flip" matmuls (lhsT = activationT slice).
z and z_dyn share one flip via rhs = [enc_w2^T | enc_w2^T K_w^T] with a
zero pad to N=256 so f32r runs at full rate; bias [enc_b2 | enc_b2 K_w^T]
is added by DVE. The two decoder hidden layers fuse z_dyn away entirely
via W = dec_w1 @ K_w. The P-step rollout uses K-power matrices
KPOW[:, 64(t-1):64t] = (K^T)^t built by doubling (lo half, then one
SBUF->SBUF DMA replicates to the hi half); x_pred staging is chunk-major
(one batch per partition -> 2KB-contiguous DRAM runs).
"""

import numpy as np

B, S, D, L, H, NCORES = 1024, 512, 32, 64, 64, 8
BS = B // NCORES          # 128 batches per core
NPAIR = BS // 2           # 64 pairs

_CACHE = {}


def _build(P, npair=NPAIR, do_main=True, do_pred=True):
    import concourse.mybir as mybir
    import concourse.tile as tile
    from concourse import bacc

    F32 = mybir.dt.float32
    F32R = mybir.dt.float32r
    AF = mybir.ActivationFunctionType
    ALU = mybir.AluOpType

    assert P % 16 == 0
    NTB = P // 16  # pred time blocks

    nc = bacc.Bacc("TRN2", target_bir_lowering=False, debug=False,
                   enable_asserts=True, num_devices=NCORES)

    xt_d = nc.dram_tensor("xt", [npair, 64, S], F32R, kind="ExternalInput").ap()
    ew1_d = nc.dram_tensor("enc_w1", [H, D], F32, kind="ExternalInput").ap()
    eb1_d = nc.dram_tensor("enc_b1", [H], F32, kind="ExternalInput").ap()
    ew2_d = nc.dram_tensor("enc_w2", [L, H], F32, kind="ExternalInput").ap()
    eb2_d = nc.dram_tensor("enc_b2", [L], F32, kind="ExternalInput").ap()
    dw1_d = nc.dram_tensor("dec_w1", [H, L], F32, kind="ExternalInput").ap()
    db1_d = nc.dram_tensor("dec_b1", [H], F32, kind="ExternalInput").ap()
    dw2_d = nc.dram_tensor("dec_w2", [D, H], F32, kind="ExternalInput").ap()
    db2_d = nc.dram_tensor("dec_b2", [D], F32, kind="ExternalInput").ap()
    kw_d = nc.dram_tensor("K_w", [L, L], F32, kind="ExternalInput").ap()

    xrec_d = nc.dram_tensor("x_rec", [2 * npair, S, D], F32, kind="ExternalOutput").ap()
    xdyn_d = nc.dram_tensor("x_dyn", [2 * npair, S, D], F32, kind="ExternalOutput").ap()
    xprd_d = nc.dram_tensor("x_pred", [2 * npair, P, D], F32, kind="ExternalOutput").ap()
    z_d = nc.dram_tensor("z", [2 * npair, S, L], F32, kind="ExternalOutput").ap()
    zdyn_d = nc.dram_tensor("z_dyn", [2 * npair, S, L], F32, kind="ExternalOutput").ap()

    LO, HI = slice(0, 64), slice(64, 128)

    def r32(ap):
        return ap.bitcast(F32R)

    with tile.TileContext(nc) as tc:
        with tc.tile_pool(name="const", bufs=1) as cp:
            # ---------------- weights / biases / constants ----------------
            # block-diagonal pair weights: [0:64,0:64]=W, [64:128,64:128]=W
            eblk = cp.tile([128, 128], F32R)   # enc_w1^T blocks at rows 0:32/32:64
            ew2blk = cp.tile([128, 128], F32R)  # enc_w2^T block-diag
            dwblk = cp.tile([128, 128], F32R)  # dec_w1^T block-diag
            wfblk = cp.tile([128, 128], F32R)  # (dec_w1 K_w)^T block-diag
            nc.gpsimd.memset(eblk[:, :].bitcast(F32), 0.0)
            nc.gpsimd.memset(ew2blk[:, :].bitcast(F32), 0.0)
            nc.gpsimd.memset(dwblk[:, :].bitcast(F32), 0.0)
            nc.gpsimd.memset(wfblk[:, :].bitcast(F32), 0.0)
            nc.sync.dma_start(eblk[0:32, 0:64], r32(ew1_d.rearrange("a b -> b a")))
            nc.sync.dma_start(eblk[32:64, 64:128], r32(ew1_d.rearrange("a b -> b a")))
            nc.sync.dma_start(ew2blk[LO, 0:64], r32(ew2_d.rearrange("a b -> b a")))
            nc.sync.dma_start(ew2blk[HI, 64:128], r32(ew2_d.rearrange("a b -> b a")))
            nc.sync.dma_start(dwblk[LO, 0:64], r32(dw1_d.rearrange("a b -> b a")))
            nc.sync.dma_start(dwblk[HI, 64:128], r32(dw1_d.rearrange("a b -> b a")))

            ew2s = cp.tile([128, H], F32R)    # enc_w2 straight (lo)
            dw1t = cp.tile([128, H], F32R)    # dec_w1^T lo/hi (pred + wf mm)
            dw2t = cp.tile([128, D], F32R)    # dec_w2^T lo/hi (flips)
            kwt = cp.tile([128, L], F32R)     # K_w^T lo
            kst = cp.tile([128, L], F32R)     # K_w straight lo
            nc.sync.dma_start(ew2s[0:64, :], r32(ew2_d[:, :]))
            nc.sync.dma_start(dw1t[LO, :], r32(dw1_d.rearrange("a b -> b a")))
            nc.sync.dma_start(dw1t[HI, :], r32(dw1_d.rearrange("a b -> b a")))
            nc.sync.dma_start(dw2t[LO, :], r32(dw2_d.rearrange("a b -> b a")))
            nc.sync.dma_start(dw2t[HI, :], r32(dw2_d.rearrange("a b -> b a")))
            nc.sync.dma_start(kwt[LO, :], r32(kw_d.rearrange("a b -> b a")))
            nc.sync.dma_start(kst[LO, :], r32(kw_d[:, :]))

            b1e = cp.tile([128, 1], F32)     # enc_b1 lo/hi (per-partition)
            b2e = cp.tile([128, 1], F32)     # enc_b2 lo/hi
            b1d = cp.tile([128, 1], F32)     # dec_b1 lo/hi
            for t_, src in ((b1e, eb1_d), (b2e, eb2_d), (b1d, db1_d)):
                nc.sync.dma_start(t_[LO, 0:1], src.rearrange("(a b) -> a b", b=1))
                nc.sync.dma_start(t_[HI, 0:1], src.rearrange("(a b) -> a b", b=1))
            b2er = cp.tile([128, 1], F32R)
            nc.vector.tensor_copy(b2er[LO, 0:1], b2e[LO, 0:1])

            # z/z_dyn flip rhs [enc_w2^T | EK | zero pad], lo+hi rows
            zzdw = cp.tile([128, 256], F32R)
            nc.gpsimd.memset(zzdw[:, :].bitcast(F32), 0.0)
            nc.sync.dma_start(zzdw[LO, 0:64], r32(ew2_d.rearrange("a b -> b a")))

            ones = cp.tile([128, 128], F32)
            nc.gpsimd.memset(ones[0:1, :], 1.0)
            zzd_row = cp.tile([128, 512], F32)  # 4x [enc_b2 | enc_b2 @ K_w^T]
            b2d_row = cp.tile([128, 512], F32)  # 16x dec_b2
            for r in range(4):
                nc.sync.dma_start(zzd_row[0:1, 128 * r:128 * r + 64],
                                  eb2_d.rearrange("(a b) -> a b", a=1))
            for r in range(16):
                nc.sync.dma_start(b2d_row[0:1, 32 * r:32 * r + 32],
                                  db2_d.rearrange("(a b) -> a b", a=1))
            zzd_rep = cp.tile([128, 512], F32)
            b2d_rep = cp.tile([128, 512], F32)
            bzrow = cp.tile([128, 64], F32)

            kpow = cp.tile([128, 64 * P], F32R)   # (K^T)^t at [:, 64(t-1):64t]
            zlast = cp.tile([128, L], F32R)       # col j = z[:,-1,:] of pair j
            qcur = cp.tile([128, L], F32R)        # K^m (doubling helper)
            qnxt = cp.tile([128, L], F32R)

            with tc.tile_pool(name="sps", bufs=2, space="PSUM") as sps:
                # EK (lo): EK[h,l'] = sum_l enc_w2[l,h] K^T[l,l']
                ekp = sps.tile([128, 512], F32, tag="s")
                nc.tensor.matmul(ekp[0:64, 0:64], ew2s[0:64, :], kwt[0:64, :],
                                 start=True, stop=True)
                nc.scalar.copy(zzdw[LO, 64:128], ekp[0:64, 0:64])
                # replicate zzdw lo rows -> hi rows (incl. zero pad)
                nc.sync.dma_start(zzdw[HI, :], zzdw[LO, :])

                # wf (lo): (dec_w1 K_w)^T = K_w^T dec_w1^T
                wfp = sps.tile([128, 512], F32, tag="s")
                nc.tensor.matmul(wfp[0:64, 0:64], kst[LO, :], dw1t[LO, :],
                                 start=True, stop=True)
                nc.scalar.copy(wfblk[LO, 0:64], wfp[0:64, 0:64])
                nc.sync.dma_start(wfblk[HI, 64:128], wfblk[LO, 0:64])

                # bzd row = enc_b2 @ K_w^T
                bzp = sps.tile([128, 512], F32, tag="s")
                nc.tensor.matmul(bzp[0:1, 0:64], b2er[LO, 0:1], kwt[LO, :],
                                 start=True, stop=True)
                nc.vector.tensor_copy(bzrow[0:1, :], bzp[0:1, 0:64])
                for r in range(4):
                    nc.vector.tensor_copy(zzd_row[0:1, 128 * r + 64:128 * r + 128],
                                          bzrow[0:1, :])

                # broadcast bias rows to all partitions via K=1 ones matmuls
                bp = sps.tile([128, 512], F32, tag="s")
                nc.tensor.matmul(bp[:, :], ones[0:1, 0:128], zzd_row[0:1, :],
                                 start=True, stop=True)
                nc.vector.tensor_copy(zzd_rep[:, :], bp[:, :])
                bp2 = sps.tile([128, 512], F32, tag="s")
                nc.tensor.matmul(bp2[:, :], ones[0:1, 0:128], b2d_row[0:1, :],
                                 start=True, stop=True)
                nc.vector.tensor_copy(b2d_rep[:, :], bp2[:, :])

                # ---------------- K powers (lo half only) ----------------
                nc.vector.tensor_copy(kpow[LO, 0:64], kwt[LO, :])  # P_1
                for t in range(1, 8):  # P_2..P_8 (serial)
                    pp = sps.tile([128, 512], F32, tag="s")
                    nc.tensor.matmul(pp[0:64, 0:64], kst[LO, :],
                                     kpow[LO, 64 * (t - 1):64 * t],
                                     start=True, stop=True)
                    nc.vector.tensor_copy(kpow[LO, 64 * t:64 * t + 64], pp[0:64, 0:64])
                nc.vector.tensor_copy(qcur[LO, :], kst[LO, :])  # Q_1
                for m in (1, 2, 4):   # Q_2, Q_4, Q_8
                    qp = sps.tile([128, 512], F32, tag="s")
                    nc.tensor.matmul(qp[0:64, 0:64],
                                     kpow[LO, 64 * (m - 1):64 * m],
                                     qcur[LO, :], start=True, stop=True)
                    nc.vector.tensor_copy(qnxt[LO, :], qp[0:64, 0:64])
                    qcur, qnxt = qnxt, qcur
                m = 8
                while m < P:  # S_{m+1..2m} = mm(lhsT=Q_m, rhs=S_{1..m})
                    for ch in range(max(1, (64 * m) // 512)):
                        n0 = 512 * ch if 64 * m > 512 else 0
                        nn = min(512, 64 * m)
                        dp = sps.tile([128, 512], F32, tag="s")
                        nc.tensor.matmul(dp[0:64, 0:nn], qcur[LO, :],
                                         kpow[LO, n0:n0 + nn],
                                         start=True, stop=True)
                        nc.vector.tensor_copy(kpow[LO, 64 * m + n0:64 * m + n0 + nn],
                                              dp[0:64, 0:nn])
                    if 2 * m < P:
                        qp = sps.tile([128, 512], F32, tag="s")
                        nc.tensor.matmul(qp[0:64, 0:64],
                                         kpow[LO, 64 * (m - 1):64 * m],
                                         qcur[LO, :], start=True, stop=True)
                        nc.vector.tensor_copy(qnxt[LO, :], qp[0:64, 0:64])
                        qcur, qnxt = qnxt, qcur
                    m *= 2
                # replicate powers to hi rows for the odd-batch rollout chain
                nc.sync.dma_start(kpow[HI, :], kpow[LO, :])

            # ---------------- main loop over batch pairs ----------------
            # PSUM budget (8 banks): mm x3, ztmA, ztmB, xpA, xpB (+1 spare)
            with tc.tile_pool(name="work", bufs=3) as wp_pool, \
                 tc.tile_pool(name="xin", bufs=8) as xin_pool, \
                 tc.tile_pool(name="stage", bufs=3) as st_pool, \
                 tc.tile_pool(name="mmp", bufs=2, space="PSUM") as mm_pool, \
                 tc.tile_pool(name="ztmp", bufs=2, space="PSUM") as ztm_pool, \
                 tc.tile_pool(name="tmp", bufs=1, space="PSUM") as tm_pool:

                for j in range(npair if do_main else 0):
                    bA = 2 * j
                    xT = xin_pool.tile([128, S], F32R, tag="xT")
                    nc.gpsimd.dma_start(xT[0:64, :], xt_d[j])

                    # encoder layer 1: block-diag, rows 0:64 -> out pair-stacked
                    pre1 = mm_pool.tile([128, S], F32, tag="mm")
                    nc.tensor.matmul(pre1[:, :], eblk[0:64, :], xT[0:64, :],
                                     start=True, stop=True)
                    h1T = wp_pool.tile([128, S], F32R, tag="h1T")
                    nc.scalar.activation(h1T[:, :], pre1[:, :], AF.Relu, bias=b1e[:, 0:1])

                    # encoder layer 2 (K=128 block-diag)
                    zp = mm_pool.tile([128, S], F32, tag="mm")
                    nc.tensor.matmul(zp[:, :], ew2blk[:, :], h1T[:, :], start=True, stop=True)
                    zT = wp_pool.tile([128, S], F32R, tag="zT")
                    nc.scalar.activation(zT[:, :], zp[:, :], AF.Identity, bias=b2e[:, 0:1])

                    # decoder hidden on z and (fused) on z_dyn, pair-stacked
                    hz = mm_pool.tile([128, S], F32, tag="mm")
                    nc.tensor.matmul(hz[:, :], dwblk[:, :], zT[:, :], start=True, stop=True)
                    h1z = wp_pool.tile([128, S], F32R, tag="h1z")
                    nc.scalar.activation(h1z[:, :], hz[:, :], AF.Relu, bias=b1d[:, 0:1])
                    hd = mm_pool.tile([128, S], F32, tag="mm")
                    nc.tensor.matmul(hd[:, :], wfblk[:, :], zT[:, :], start=True, stop=True)
                    h1d = wp_pool.tile([128, S], F32R, tag="h1d")
                    nc.scalar.activation(h1d[:, :], hd[:, :], AF.Relu, bias=b1d[:, 0:1])

                    # ---- flip-mms (token-major, all dst partition 0) ----
                    # z/zdyn merged; chunks 0..2 zero-padded to N=256 (f32r full rate)
                    ztmA = ztm_pool.tile([128, 512], F32, tag="ztmA")
                    ztmB = ztm_pool.tile([128, 512], F32, tag="ztmB")
                    for c in range(4):
                        cs = slice(128 * c, 128 * c + 128)
                        nn = 256 if c < 3 else 128
                        nc.tensor.matmul(ztmA[:, 128 * c:128 * c + nn],
                                         h1T[LO, cs], zzdw[LO, 0:nn], start=True, stop=True)
                        nc.tensor.matmul(ztmB[:, 128 * c:128 * c + nn],
                                         h1T[HI, cs], zzdw[HI, 0:nn], start=True, stop=True)
                    # x_rec/x_dyn flips: bank per batch (A: rows 0:64, B: rows 64:128)
                    xpA = tm_pool.tile([128, 512], F32, tag="xpA")  # use [:, 0:256]
                    xpB = tm_pool.tile([128, 512], F32, tag="xpB")
                    for c in range(4):
                        cs = slice(128 * c, 128 * c + 128)
                        nc.tensor.matmul(xpA[:, 32 * c:32 * c + 32],
                                         h1z[LO, cs], dw2t[LO, :], start=True, stop=True)
                        nc.tensor.matmul(xpA[:, 128 + 32 * c:128 + 32 * c + 32],
                                         h1d[LO, cs], dw2t[LO, :], start=True, stop=True)
                        nc.tensor.matmul(xpB[:, 32 * c:32 * c + 32],
                                         h1z[HI, cs], dw2t[HI, :], start=True, stop=True)
                        nc.tensor.matmul(xpB[:, 128 + 32 * c:128 + 32 * c + 32],
                                         h1d[HI, cs], dw2t[HI, :], start=True, stop=True)

                    # bias adds into 2-pair staging tiles (stores batched
                    # over 4 batches to halve DMA count). Layouts put the
                    # output-tensor split OUTERMOST so DMA APs stay <=3 dims:
                    # zs2: [z (q c l) | zdyn (q c l)], xs4: [rec (q c d) | dyn ...]
                    GRP = 4  # pairs per store batch
                    if j % GRP == 0:
                        zs2 = st_pool.tile([128, 1024 * GRP], F32, tag="zs2")
                        xs4 = st_pool.tile([128, 512 * GRP], F32, tag="xs4")
                    zv2 = zs2[:, :].rearrange("p (w q c l) -> p w q c l", w=2, q=2 * GRP, c=4)
                    xv4 = xs4[:, :].rearrange("p (o q c d) -> p o q c d", o=2, q=2 * GRP, c=4)
                    qA, qB = 2 * (j % GRP), 2 * (j % GRP) + 1
                    nc.vector.tensor_tensor(
                        zv2[:, :, qA], ztmA[:, :].rearrange("p (c w l) -> p w c l", c=4, w=2),
                        zzd_rep[:, :].rearrange("p (c w l) -> p w c l", c=4, w=2), op=ALU.add)
                    nc.vector.tensor_tensor(
                        zv2[:, :, qB], ztmB[:, :].rearrange("p (c w l) -> p w c l", c=4, w=2),
                        zzd_rep[:, :].rearrange("p (c w l) -> p w c l", c=4, w=2), op=ALU.add)
                    nc.vector.tensor_tensor(
                        xv4[:, :, qA], xpA[:, 0:256].rearrange("p (o cd) -> p o cd", o=2),
                        b2d_rep[:, 0:256].rearrange("p (o cd) -> p o cd", o=2), op=ALU.add)
                    nc.vector.tensor_tensor(
                        xv4[:, :, qB], xpB[:, 0:256].rearrange("p (o cd) -> p o cd", o=2),
                        b2d_rep[:, 0:256].rearrange("p (o cd) -> p o cd", o=2), op=ALU.add)

                    # keep z[:, -1, :] column for the rollout
                    nc.vector.tensor_copy(zlast[:, j:j + 1], zT[:, S - 1:S])

                    # ---- stores (every second pair, 4 batches per DMA) ----
                    if j % GRP == GRP - 1 or j == npair - 1:
                        b0 = bA - 2 * (j % GRP)
                        nb = 2 * (j % GRP) + 2
                        nc.sync.dma_start(
                            z_d[b0:b0 + nb].rearrange("q (c p) l -> p q c l", p=128),
                            zs2[:, 0:256 * nb].rearrange("p (q c l) -> p q c l", q=nb, c=4))
                        nc.scalar.dma_start(
                            zdyn_d[b0:b0 + nb].rearrange("q (c p) l -> p q c l", p=128),
                            zs2[:, 512 * GRP:512 * GRP + 256 * nb].rearrange("p (q c l) -> p q c l", q=nb, c=4))
                        nc.sync.dma_start(
                            xrec_d[b0:b0 + nb].rearrange("q (c p) d -> p q c d", p=128),
                            xs4[:, 0:128 * nb].rearrange("p (q c d) -> p q c d", q=nb, c=4))
                        nc.scalar.dma_start(
                            xdyn_d[b0:b0 + nb].rearrange("q (c p) d -> p q c d", p=128),
                            xs4[:, 256 * GRP:256 * GRP + 128 * nb].rearrange("p (q c d) -> p q c d", q=nb, c=4))

            # ---------------- prediction rollout ----------------
            # chain A = even batches (zlast rows 0:64, kpow lo),
            # chain B = odd batches (rows 64:128, kpow hi); all dst partition 0.
            with tc.tile_pool(name="pwork", bufs=2) as pw_pool, \
                 tc.tile_pool(name="pmm", bufs=1, space="PSUM") as pmm_pool, \
                 tc.tile_pool(name="ph", bufs=1, space="PSUM") as ph_pool, \
                 tc.tile_pool(name="pacc", bufs=2, space="PSUM") as pacc_pool:
                xpr_v = xprd_d.rearrange("(b2 two) t d -> b2 two t d", two=2)
                npq = npair  # pair count = flip M
                for tb in range(NTB if do_pred else 0):
                    xppA = pacc_pool.tile([128, 512], F32, tag="xppA")
                    xppB = pacc_pool.tile([128, 512], F32, tag="xppB")
                    if npair < 64:
                        nc.vector.memset(xppA[:, :], 0.0)
                        nc.vector.memset(xppB[:, :], 0.0)
                    for sb in range(2):
                        t0 = 16 * tb + 8 * sb  # powers t0+1 .. t0+8
                        # zpred for 8 steps: 4 t-pair matmuls (M=128) per chain
                        zppA = pmm_pool.tile([128, 512], F32, tag="zppA")
                        zppB = pmm_pool.tile([128, 512], F32, tag="zppB")
                        if npair < 64:
                            nc.vector.memset(zppA[:, :], 0.0)
                            nc.vector.memset(zppB[:, :], 0.0)
                        for q in range(4):
                            ks = slice(64 * (t0 + 2 * q), 64 * (t0 + 2 * q) + 128)
                            nc.tensor.matmul(zppA[:, 64 * q:64 * q + npq],
                                             kpow[LO, ks], zlast[LO, 0:npq],
                                             start=True, stop=True)
                            nc.tensor.matmul(zppB[:, 64 * q:64 * q + npq],
                                             kpow[HI, ks], zlast[HI, 0:npq],
                                             start=True, stop=True)
                        # partitions of zpp: 0:64 = t-even l, 64:128 = t-odd l
                        zpsA = pw_pool.tile([128, 256], F32R, tag="zpsA")
                        zpsB = pw_pool.tile([128, 256], F32R, tag="zpsB")
                        nc.vector.tensor_copy(zpsA[:, :], zppA[:, 0:256])
                        nc.vector.tensor_copy(zpsB[:, :], zppB[:, 0:256])
                        # decoder hidden: parity e reads rows 0:64, parity o rows 64:128
                        hE = ph_pool.tile([128, 512], F32, tag="hE")
                        hO = ph_pool.tile([128, 512], F32, tag="hO")
                        nc.tensor.matmul(hE[0:64, 0:256], dw1t[LO, :], zpsA[LO, :],
                                         start=True, stop=True)
                        nc.tensor.matmul(hO[0:64, 0:256], dw1t[HI, :], zpsA[HI, :],
                                         start=True, stop=True)
                        nc.tensor.matmul(hE[0:64, 256:512], dw1t[LO, :], zpsB[LO, :],
                                         start=True, stop=True)
                        nc.tensor.matmul(hO[0:64, 256:512], dw1t[HI, :], zpsB[HI, :],
                                         start=True, stop=True)
                        h1E = pw_pool.tile([128, 512], F32R, tag="h1E")
                        h1O = pw_pool.tile([128, 512], F32R, tag="h1O")
                        nc.scalar.activation(h1E[0:64, :], hE[0:64, :], AF.Relu, bias=b1d[LO, 0:1])
                        nc.scalar.activation(h1O[0:64, :], hO[0:64, :], AF.Relu, bias=b1d[LO, 0:1])
                        # flips: t = t0 + 2q + par; chain A cols 0:256, B cols 256:512
                        for q in range(4):
                            for par, h1 in ((0, h1E), (1, h1O)):
                                tloc = 8 * sb + 2 * q + par
                                nc.tensor.matmul(
                                    xppA[0:npq, 32 * tloc:32 * tloc + 32],
                                    h1[0:64, 64 * q:64 * q + npq], dw2t[LO, :],
                                    start=True, stop=True)
                                nc.tensor.matmul(
                                    xppB[0:npq, 32 * tloc:32 * tloc + 32],
                                    h1[0:64, 256 + 64 * q:256 + 64 * q + npq], dw2t[LO, :],
                                    start=True, stop=True)
                    xpsA = pw_pool.tile([128, 512], F32, tag="xpsA")
                    xpsB = pw_pool.tile([128, 512], F32, tag="xpsB")
                    nc.vector.tensor_tensor(xpsA[0:64, :], xppA[0:64, :], b2d_rep[0:64, :], op=ALU.add)
                    nc.vector.tensor_tensor(xpsB[0:64, :], xppB[0:64, :], b2d_rep[0:64, :], op=ALU.add)
                    nc.sync.dma_start(
                        xpr_v[:, 0, 16 * tb:16 * tb + 16, :],
                        xpsA[0:npq, :].rearrange("p (t d) -> p t d", t=16))
                    nc.scalar.dma_start(
                        xpr_v[:, 1, 16 * tb:16 * tb + 16, :],
                        xpsB[0:npq, :].rearrange("p (t d) -> p t d", t=16))

    nc.compile()
    return nc


def _prep_xt(x_shard):
    # [128, 512, 32] -> [64 pairs, 64, 512]; rows 0:32 even batch x^T, 32:64 odd
    xt = np.empty((NPAIR, 64, S), np.float32)
    xt[:, 0:32, :] = x_shard[0::2].transpose(0, 2, 1)
    xt[:, 32:64, :] = x_shard[1::2].transpose(0, 2, 1)
    return xt


def _run(inputs, trace=False):
    from concourse import bass_utils

    x = np.asarray(inputs["x"], np.float32)
    P = int(inputs["pred_len"])
    key = P
    if key not in _CACHE:
        _CACHE[key] = _build(P)
    nc = _CACHE[key]

    weights = {k: np.ascontiguousarray(np.asarray(inputs[k], np.float32))
               for k in ("enc_w1", "enc_b1", "enc_w2", "enc_b2",
                         "dec_w1", "dec_b1", "dec_w2", "dec_b2", "K_w")}
    in_maps = []
    for c in range(NCORES):
        m = dict(weights)
        m["xt"] = _prep_xt(x[c * BS:(c + 1) * BS])
        in_maps.append(m)

    res = bass_utils.run_bass_kernel_spmd(nc, in_maps, core_ids=list(range(NCORES)),
                                          trace=trace)
    rs = res.results
    x_rec = np.concatenate([r["x_rec"] for r in rs], 0)
    x_dyn = np.concatenate([r["x_dyn"] for r in rs], 0)
    x_pred = np.concatenate([r["x_pred"] for r in rs], 0)
    z = np.concatenate([r["z"] for r in rs], 0)
    z_dyn = np.concatenate([r["z_dyn"] for r in rs], 0)
    return (x_rec, x_dyn, x_pred, z, z_dyn), res


def kernel(**inputs):
    return _run(inputs)[0]
